# revision 16
# baseline (speedup 1.0000x reference)
"""Trainium2 Bass kernel for nn_MultiHeadLiftLayer (GNN edge-signal lift).

Computes, for each edge e with endpoints (s, t):
    out[e, k] = relu( x[s] . a_src[k] + x[t] . a_tgt[k] ),  k = 0..3

Architecture (v5, "rank-major expansion + single-side gather"):

The previous (baseline) kernel gathered both endpoints' x rows per edge
via SWDGE dma_gather; its trace shows the bottleneck is the Q7
descriptor-generation loop on the Pool engine (~2.5us per 896-idx call,
strictly serial) -- NOT DMA bandwidth. This version removes the entire
src side from the Q7 path:

  - Phase A (on device): p[n] = [x[n].a_src | x[n].a_tgt]  (8 f16 =
    16B per node) is computed by 391 node-major PE matmuls (lhsT =
    xT 128-node chunk, rhs = the 64x8 attention pack), cast f16 by the
    ACT engine, and stored to HBM as `p_plain` (contiguous 16B rows in
    a host-chosen node order) plus a 256B-strided copy `ptab` for the
    gather (SWDGE non-transpose gather requires a 256B-multiple row
    stride; element itself is 16B -- the bass-level 256B elem_size
    assert only applies to transpose mode, so we emit the instruction
    directly).
  - Node order ñ = per-core src-degree descending. Edge e is assigned
    slot (k = rank of e within its src node, u = ñ(s_e)). Slots are
    laid out rank-major: segment k holds nodes u < n_k (a PREFIX of ñ
    since ñ is degree-sorted). The src-side values for segment k are
    therefore a contiguous prefix of p_plain -- ONE affine 3-dim DMA
    per segment (no per-edge work at all).
  - The tgt side is the only per-edge gather: 16B rows from ptab via
    non-transpose dma_gather (idx j lands at partition j%128, word
    j//128 -- verified on HW). int16 indices cover all 50048 rows via a
    signed bias: the table AP is based at row 32768 and indices are
    ñ(t)-32768 (negative indices verified to address below the base on
    HW; CoreSim has an over-strict >=-1 assert, so sim mode is not
    supported for this kernel).
  - Calls carry 896 positions; position j=895 (slot r=895) is a
    structural pad so the trailing index of every call is >= 0 (the Q7
    ucode trims trailing negatives). A call covers 895 slots; slot r
    sits at (partition r//7, 16B-word r%7); gather position j =
    (r%7)*128 + r//7.
  - Combine: out = relu(DS[.., 0:4] + DT[.., 4:8]) with full
    128-partition parallelism (DVE add + ACT relu), f16, then one DMA
    per segment to HBM.
  - Edges that overflow the fixed per-segment capacities (src-rank >= 6,
    ~300 per core, or a segment fuller than mean+~8 sigma) go to 2
    fixup calls where BOTH endpoints are gathered; the fixup gathers are
    scheduled first so their combine chain hides under the main calls.
  - Gather instructions carry no explicit queue-spacing deps: the Q7
    ucode's own descriptor-ring await_space handles backpressure, so
    desc-gen runs at the serial Q7 floor (~2.0us/call) while the SDMA
    engines drain concurrently.

Measured: 409us vs 654us baseline (same rel err 5.7e-04). Remaining
profile: ~140us head (xT in + phase A + the 50K-descriptor respread --
fabric-bound, must complete before any gather), ~250us tgt-gather window
(Q7 desc-gen floor), ~20us tail.
"""

import numpy as np

import concourse.ap_utils as ap_utils
import concourse.bacc as bacc
import concourse.bass as bass
import concourse.mybir as mybir
import concourse.tile as tile
from concourse.bass_utils import run_bass_kernel_spmd
from concourse.instruction_name_ordered_set import InstructionNameOrderedSet

# ---- problem constants (hardcoded per contract) ----
N_NODES = 50000
N_EDGES = 800000
F_IN = 64
K = 4
CORES = 8

NP = 50048                 # padded node count = 128 * 391
NCH = 391                  # node chunks of 128 (phase A matmuls)
BIAS = 32768               # idx bias: table AP based at row 32768
CALL = 896                 # gather positions per call
USE = 895                  # usable slots per call (pos 895 = pad)
WPP = 7                    # 16B words per partition per call (896/128)

# fixed per-segment call capacities, k = 0..5 (per-core degrees are
# ~Poisson(2); capacities are mean + >=6.6 sigma). Edges with src-rank
# >= 6 (~300 per core) go to the fixup calls.
CALLS_K = [49, 34, 19, 9, 4, 2]
KMAX = len(CALLS_K)
FIX_CALLS = 2              # fixup slot-calls (each needs 2 gathers)
N_MAIN = sum(CALLS_K)      # 123 main (tgt-gather) calls
N_SLOT_CALLS = N_MAIN + FIX_CALLS          # 127 slot-calls
N_GATHER = N_MAIN + 2 * FIX_CALLS          # 131 gather instructions
ICOLS = CALL // 16         # 56 idx columns per call (wrapped layout)

F32 = mybir.dt.float32
F16 = mybir.dt.float16
I16 = mybir.dt.int16

_PROGRAM_CACHE = {}


def _dma_gather_small(g, out_ap, in_ap, idxs_ap, num_idxs, elem_size,
                      elem_step, queue_num):
    """nc.gpsimd.dma_gather, non-transpose, without the 256B elem_size
    assert (which is a transpose-mode restriction; HW decode only
    requires the row stride to be a 256B multiple)."""
    g._assert_queue_num(queue_num)
    assert idxs_ap.dtype == mybir.dt.int16
    assert in_ap.dtype == out_ap.dtype
    assert ap_utils.ap_is_contiguous(in_ap.ap[1:])
    assert ap_utils.ap_is_contiguous(out_ap.ap[1:])
    assert ap_utils.ap_is_contiguous(idxs_ap.ap[1:])
    assert in_ap.ap[-1][1] == out_ap.ap[-1][1] == elem_size
    assert out_ap.ap[0][1] * out_ap.ap[1][1] == CALL
    assert in_ap.ap[0][0] == elem_step
    stride_bytes = elem_step * mybir.dt.size(in_ap.dtype)
    stride_bytes_256 = stride_bytes // 256
    assert stride_bytes_256 * 256 == stride_bytes and stride_bytes_256 < 256
    _in_ap = g.lower_ap_dma(in_ap, for_custom_bir_dma=True)
    _idxs_ap = g.lower_ap(idxs_ap)
    _out_ap = g.lower_ap(out_ap)
    return g.add_instruction(
        mybir.InstDMAGatherAnt(
            name=g.bass.get_next_instruction_name(),
            ins=[*_in_ap, _idxs_ap,
                 g.lower_val_access(g.to_reg(num_idxs))],
            outs=[_out_ap],
            transpose=False,
            num_idxs=num_idxs,
            elem_size=elem_size,
            stride_bytes_256=stride_bytes_256,
            gen_mode=0,
            single_packet=True,
            queue_num=queue_num,
        )
    )


def _build_program():
    nc = bacc.Bacc("TRN2", num_swdge_queues=4)

    xT_in = nc.dram_tensor("xT_in", [F_IN, NP], F16, kind="ExternalInput")
    a_in = nc.dram_tensor("a_in", [F_IN, 8], F16, kind="ExternalInput")
    idx_in = nc.dram_tensor("idx_in", [128, N_GATHER * ICOLS], I16,
                            kind="ExternalInput")
    out_d = nc.dram_tensor("out", [128, N_SLOT_CALLS * WPP * K], F16,
                           kind="ExternalOutput")
    p_plain = nc.dram_tensor("p_plain", [128, NCH * 8], F16, kind="Internal")
    ptab = nc.dram_tensor("ptab", [NP, 128], F16, kind="Internal")

    # segment -> (first main call index, ncalls)
    seg_base = []
    b = 0
    for k in range(KMAX):
        seg_base.append(b)
        b += CALLS_K[k]

    with tile.TileContext(nc) as tc:
        with (
            tc.tile_pool(name="const", bufs=1) as cpool,
            tc.tile_pool(name="ps", bufs=2, space="PSUM") as ppool,
            tc.tile_pool(name="seg", bufs=1) as spool,
        ):
            a_raw = cpool.tile([F_IN, 8], F16)
            nc.sync.dma_start(out=a_raw[:], in_=a_in[:])
            a_sb = cpool.tile([F_IN, 8], F16)
            nc.vector.tensor_copy(out=a_sb[:], in_=a_raw[:])
            idx = cpool.tile([128, N_GATHER * ICOLS], I16)
            nc.sync.dma_start(out=idx[:], in_=idx_in[:])
            # xT arrives per-supertile so matmuls/casts/respreads pipeline
            xt = cpool.tile([F_IN, NP], F16)
            xt_done = 0
            while xt_done < NCH:
                m = min(64, NCH - xt_done)
                nc.sync.dma_start(
                    out=xt[:, 128 * xt_done:128 * (xt_done + m)],
                    in_=xT_in[:, 128 * xt_done:128 * (xt_done + m)])
                xt_done += m

            # ---- Phase A: p = [x.a_src | x.a_tgt] per node ----
            # Per 64-chunk supertile: matmuls -> f16 cast -> (a) write to
            # p_plain (contiguous, 128 descs) and (b) respread straight
            # into ptab's 256B-strided rows. The respreads (50K 16B
            # descriptors total) pipeline under the remaining matmuls
            # instead of serializing before the gathers.
            stage = cpool.tile([128, NCH * 8], F16)
            done = 0
            while done < NCH:
                m = min(64, NCH - done)
                ps = ppool.tile([128, 8 * m], F32)
                for i in range(m):
                    c = done + i
                    nc.tensor.matmul(
                        out=ps[:, 8 * i:8 * i + 8],
                        lhsT=xt[:, 128 * c:128 * c + 128],
                        rhs=a_sb[:, 0:8],
                        start=True,
                        stop=True,
                    )
                sl = stage[:, 8 * done:8 * (done + m)]
                nc.scalar.copy(out=sl, in_=ps[:, 0:8 * m])
                nc.sync.dma_start(
                    out=bass.AP(p_plain, 8 * done, [[NCH * 8, 128], [1, 8 * m]]),
                    in_=sl)
                # ptab rows ñ = p*391 + done + c', c' < m. The respread is
                # HWDGE desc-gen bound (50K 16B descriptors); split each
                # supertile's write across the scalar and vector HWDGE
                # rings so two desc-gen engines run in parallel.
                h = m // 2
                for eng, c0, cm in ((nc.scalar, 0, h), (nc.sync, h, m - h)):
                    if cm > 0:
                        eng.dma_start(
                            out=bass.AP(ptab, 128 * (done + c0),
                                        [[NCH * 128, 128], [128, cm], [1, 8]]),
                            in_=sl.rearrange("p (c e) -> p c e", e=8)
                                  [:, c0:c0 + cm, :])
                done += m

            # ---- segment tiles + src-side affine expansion ----
            ds_tiles, dt_tiles, ad_tiles, o_tiles = [], [], [], []
            for k in range(KMAX):
                ncal = CALLS_K[k]
                dst = spool.tile([128, ncal * WPP * 8], F16, tag=f"ds{k}")
                dtt = spool.tile([128, ncal * WPP * 8], F16, tag=f"dt{k}")
                adt = spool.tile([128, ncal * WPP * K], F16, tag=f"ad{k}")
                ott = spool.tile([128, ncal * WPP * K], F16, tag=f"o{k}")
                ds_tiles.append(dst)
                dt_tiles.append(dtt)
                ad_tiles.append(adt)
                o_tiles.append(ott)
                # src AP: (p: 7 slots = 56 elems, call: 895 slots = 7160
                # elems, run: 56 elems) over p_plain's flat [NP*8] f16
                src = bass.AP(p_plain, 0,
                              [[56, 128], [7160, ncal], [1, 56]])
                dsv = dst[:].rearrange("p (cl e) -> p cl e", e=56)
                nc.sync.dma_start(out=dsv, in_=src)
            # fixup tiles
            dsf = spool.tile([128, FIX_CALLS * WPP * 8], F16, tag="dsf")
            dtf = spool.tile([128, FIX_CALLS * WPP * 8], F16, tag="dtf")
            adf = spool.tile([128, FIX_CALLS * WPP * K], F16, tag="adf")
            of = spool.tile([128, FIX_CALLS * WPP * K], F16, tag="of")

            # ---- tgt-side (and fixup src) gathers ----
            tab_ap = ptab[BIAS:, 0:8]
            all_g = []

            def gather(dst_tile, call_local, gidx):
                o = dst_tile[:, call_local * 56:(call_local + 1) * 56]
                gi = _dma_gather_small(
                    nc.gpsimd,
                    out_ap=o.rearrange("p (o m) -> p o m", o=WPP),
                    in_ap=tab_ap,
                    idxs_ap=idx[:, gidx * ICOLS:(gidx + 1) * ICOLS],
                    num_idxs=CALL,
                    elem_size=8,
                    elem_step=128,
                    queue_num=len(all_g) % 4,
                )
                if all_g:
                    ns = InstructionNameOrderedSet()
                    ns.add(all_g[-1].ins.name)
                    gi.ins.add_nosync_dependencies_from(ns)
                all_g.append(gi)

            # fixup gathers FIRST so their combine chain overlaps the main
            # gathers instead of trailing the whole kernel
            gidx = N_MAIN
            for cl in range(FIX_CALLS):      # fixup src gathers
                gather(dsf, cl, gidx)
                gidx += 1
            for cl in range(FIX_CALLS):      # fixup tgt gathers
                gather(dtf, cl, gidx)
                gidx += 1
            gidx = 0
            for k in range(KMAX):
                for cl in range(CALLS_K[k]):
                    gather(dt_tiles[k], cl, gidx)
                    gidx += 1

            # ---- combine: relu(DS[..,0:4] + DT[..,4:8]) ----
            def combine(dst, dtt, ad, ot, ncal):
                n_sl = ncal * WPP
                v0 = dst[:].rearrange("p (s e) -> p s e", e=8)[:, :, 0:4]
                v1 = dtt[:].rearrange("p (s e) -> p s e", e=8)[:, :, 4:8]
                av = ad[:].rearrange("p (s e) -> p s e", e=4)
                nc.vector.tensor_add(out=av, in0=v0, in1=v1)
                nc.scalar.activation(
                    out=ot[:], in_=ad[:],
                    func=mybir.ActivationFunctionType.Relu)

            for k in range(KMAX):
                combine(ds_tiles[k], dt_tiles[k], ad_tiles[k], o_tiles[k],
                        CALLS_K[k])
            combine(dsf, dtf, adf, of, FIX_CALLS)

            # ---- output DMAs ----
            col = 0
            for k in range(KMAX):
                w = CALLS_K[k] * WPP * K
                nc.sync.dma_start(out=out_d[:, col:col + w],
                                  in_=o_tiles[k][:])
                col += w
            w = FIX_CALLS * WPP * K
            nc.sync.dma_start(out=out_d[:, col:col + w], in_=of[:])

    # pin each gather's SWDGE queue to its scheduled completion-sem lane
    from concourse.tile_sem_assignment import PROC_NAME_TO_IDX
    lane_of = {PROC_NAME_TO_IDX[f"DMASW{i}"]: i for i in range(8)}
    for blk in nc.main_func.blocks:
        for inst in blk.instructions:
            if isinstance(inst, mybir.InstDMAGatherAnt):
                lane = lane_of.get(inst.bass_scheduled_proc)
                if lane is not None:
                    inst.queue_num = lane % 4

    nc.compile()
    return nc


def get_program():
    if "nc" not in _PROGRAM_CACHE:
        _PROGRAM_CACHE["nc"] = _build_program()
    return _PROGRAM_CACHE["nc"]


def _wrap_idx(vals):
    """Wrap a length-CALL idx vector for SWDGE: pos j -> [16g + j%16,
    j//16], replicated across the 8 gpsimd cores."""
    w = vals.reshape(ICOLS, 16).T.astype(np.int16)
    return np.tile(w, (8, 1))


def prepare_core(s, t, x16, att16):
    """Host marshaling for one core: node ordering, slot assignment,
    gather indices, input tensors, and the slot->edge output map."""
    E_c = len(s)
    d = np.bincount(s, minlength=N_NODES)
    order = np.argsort(-d, kind="stable")          # ñ -> orig node id
    rank_of = np.empty(N_NODES, dtype=np.int64)
    rank_of[order] = np.arange(N_NODES)

    # xT: node with ñ-rank u -> column 128*(u%391) + u//391, so that
    # p_plain row ñ (= p*391 + c for stage partition p chunk c) == u
    xT = np.zeros((F_IN, NP), dtype=np.float16)
    uu = np.arange(N_NODES)
    cols = 128 * (uu % NCH) + (uu // NCH)
    xT[:, cols] = x16[order].T                     # x rows in ñ order

    # per-edge src rank k
    o = np.argsort(s, kind="stable")
    so = s[o]
    starts = np.searchsorted(so, so)               # first pos of value
    kk = np.empty(E_c, dtype=np.int64)
    kk[o] = np.arange(E_c) - starts
    u = rank_of[s]
    tv = rank_of[t]

    # slot assignment
    call_no = np.full(E_c, -1, dtype=np.int64)
    r_no = np.full(E_c, -1, dtype=np.int64)
    seg_base = np.cumsum([0] + CALLS_K[:-1])
    ok = kk < KMAX
    capn = np.array([CALLS_K[k] * USE for k in range(KMAX)])
    ok &= u < capn[np.clip(kk, 0, KMAX - 1)]
    call_no[ok] = seg_base[kk[ok]] + u[ok] // USE
    r_no[ok] = u[ok] % USE
    fix = np.where(~ok)[0]
    if len(fix) > FIX_CALLS * USE:
        raise RuntimeError(f"fixup overflow: {len(fix)}")
    fpos = np.arange(len(fix))
    call_no[fix] = N_MAIN + fpos // USE
    r_no[fix] = fpos % USE

    # gather position j = (r%7)*128 + r//7
    j_no = (r_no % WPP) * 128 + r_no // WPP

    # gather idx array [128, N_GATHER*ICOLS]
    idx_arr = np.zeros((128, N_GATHER * ICOLS), dtype=np.int16)
    fixe = np.where(call_no >= N_MAIN)[0]

    tgt_vals = np.zeros((N_SLOT_CALLS, CALL), dtype=np.int64)
    tgt_vals[call_no, j_no] = tv - BIAS
    src_vals = np.zeros((FIX_CALLS, CALL), dtype=np.int64)
    src_vals[call_no[fixe] - N_MAIN, j_no[fixe]] = u[fixe] - BIAS

    g = 0
    for ci in range(N_MAIN):
        idx_arr[:, g * ICOLS:(g + 1) * ICOLS] = _wrap_idx(tgt_vals[ci])
        g += 1
    for ci in range(FIX_CALLS):
        idx_arr[:, g * ICOLS:(g + 1) * ICOLS] = _wrap_idx(src_vals[ci])
        g += 1
    for ci in range(FIX_CALLS):
        idx_arr[:, g * ICOLS:(g + 1) * ICOLS] = _wrap_idx(
            tgt_vals[N_MAIN + ci])
        g += 1

    # attention pack [64, 8]
    a = np.empty((F_IN, 8), dtype=np.float16)
    a[:, :K] = att16[:, :F_IN].T
    a[:, K:] = att16[:, F_IN:].T

    in_map = {"xT_in": xT, "a_in": a, "idx_in": idx_arr}
    # output location per edge: out_d[r//7, call*28 + (r%7)*4 + k]
    out_row = r_no // WPP
    out_col = call_no * (WPP * K) + (r_no % WPP) * K
    return in_map, out_row, out_col


def prepare_passes(x, edge_index, att):
    x16 = np.asarray(x, dtype=np.float32).astype(np.float16)
    att16 = np.asarray(att, dtype=np.float32).astype(np.float16)
    ei = np.asarray(edge_index).astype(np.int64)
    E_c = N_EDGES // CORES
    in_maps, maps = [], []
    for c in range(CORES):
        sl = slice(c * E_c, (c + 1) * E_c)
        # x16 rows must be passed in ñ order: prepare_core handles the
        # permutation internally via rank_of -> pass orig-order x
        im, orow, ocol = prepare_core(ei[0, sl], ei[1, sl], x16, att16)
        in_maps.append(im)
        maps.append((orow, ocol))
    return in_maps, maps


TRACE = False
LAST_RESULTS = []


def kernel(x, edge_index, att):
    nc = get_program()
    in_maps, maps = prepare_passes(x, edge_index, att)
    LAST_RESULTS.clear()
    res = run_bass_kernel_spmd(
        nc, in_maps, core_ids=list(range(CORES)), trace=TRACE)
    LAST_RESULTS.append(res)
    E_c = N_EDGES // CORES
    out = np.empty((N_EDGES, K), dtype=np.float32)
    for c in range(CORES):
        o = np.asarray(res.results[c]["out"])    # [128, cols] f16
        orow, ocol = maps[c]
        vals = o[orow[:, None], ocol[:, None] + np.arange(K)[None, :]]
        out[c * E_c:(c + 1) * E_c] = vals.astype(np.float32)
    return out


# revision 17
# speedup vs baseline: 1.1463x; 1.1463x over previous
"""Trainium2 Bass kernel for nn_MultiHeadLiftLayer (GNN edge-signal lift).

Computes, for each edge e with endpoints (s, t):
    out[e, k] = relu( x[s] . a_src[k] + x[t] . a_tgt[k] ),  k = 0..3

Architecture (v5, "rank-major expansion + single-side gather"):

The previous (baseline) kernel gathered both endpoints' x rows per edge
via SWDGE dma_gather; its trace shows the bottleneck is the Q7
descriptor-generation loop on the Pool engine (~2.5us per 896-idx call,
strictly serial) -- NOT DMA bandwidth. This version removes the entire
src side from the Q7 path:

  - Phase A (on device): p[n] = [x[n].a_src | x[n].a_tgt]  (8 f16 =
    16B per node) is computed by 391 node-major PE matmuls (lhsT =
    xT 128-node chunk, rhs = the 64x8 attention pack), cast f16 by the
    ACT engine, and stored to HBM as `p_plain` (contiguous 16B rows in
    a host-chosen node order) plus a 256B-strided copy `ptab` for the
    gather (SWDGE non-transpose gather requires a 256B-multiple row
    stride; element itself is 16B -- the bass-level 256B elem_size
    assert only applies to transpose mode, so we emit the instruction
    directly).
  - Node order ñ = per-core src-degree descending. Edge e is assigned
    slot (k = rank of e within its src node, u = ñ(s_e)). Slots are
    laid out rank-major: segment k holds nodes u < n_k (a PREFIX of ñ
    since ñ is degree-sorted). The src-side values for segment k are
    therefore a contiguous prefix of p_plain -- ONE affine 3-dim DMA
    per segment (no per-edge work at all).
  - The tgt side is the only per-edge gather: 16B rows from ptab via
    non-transpose dma_gather (idx j lands at partition j%128, word
    j//128 -- verified on HW). int16 indices cover all 50048 rows via a
    signed bias: the table AP is based at row 32768 and indices are
    ñ(t)-32768 (negative indices verified to address below the base on
    HW; CoreSim has an over-strict >=-1 assert, so sim mode is not
    supported for this kernel).
  - Calls carry 896 positions; position j=895 (slot r=895) is a
    structural pad so the trailing index of every call is >= 0 (the Q7
    ucode trims trailing negatives). A call covers 895 slots; slot r
    sits at (partition r//7, 16B-word r%7); gather position j =
    (r%7)*128 + r//7.
  - Combine: out = relu(DS[.., 0:4] + DT[.., 4:8]) with full
    128-partition parallelism (DVE add + ACT relu), f16, then one DMA
    per segment to HBM.
  - Edges that overflow the fixed per-segment capacities (src-rank >= 6,
    ~300 per core, or a segment fuller than mean+~8 sigma) go to 2
    fixup calls where BOTH endpoints are gathered; the fixup gathers are
    scheduled first so their combine chain hides under the main calls.
  - Gather instructions carry no explicit queue-spacing deps: the Q7
    ucode's own descriptor-ring await_space handles backpressure, so
    desc-gen runs at the serial Q7 floor (~2.0us/call) while the SDMA
    engines drain concurrently.

Measured: 409us vs 654us baseline (same rel err 5.7e-04). Remaining
profile: ~140us head (xT in + phase A + the 50K-descriptor respread --
fabric-bound, must complete before any gather), ~250us tgt-gather window
(Q7 desc-gen floor), ~20us tail.
"""

import numpy as np

import concourse.ap_utils as ap_utils
import concourse.bacc as bacc
import concourse.bass as bass
import concourse.mybir as mybir
import concourse.tile as tile
from concourse.bass_utils import run_bass_kernel_spmd
from concourse.instruction_name_ordered_set import InstructionNameOrderedSet

# ---- problem constants (hardcoded per contract) ----
N_NODES = 50000
N_EDGES = 800000
F_IN = 64
K = 4
CORES = 8

NP = 50048                 # padded node count = 128 * 391
NCH = 391                  # node chunks of 128 (phase A matmuls)
BIAS = 32768               # idx bias: table AP based at row 32768
CALL = 896                 # gather positions per call
USE = 895                  # usable slots per call (pos 895 = pad)
WPP = 7                    # 16B words per partition per call (896/128)

# fixed per-segment call capacities, k = 0..5 (per-core degrees are
# ~Poisson(2); capacities are mean + >=6.6 sigma). Edges with src-rank
# >= 6 (~300 per core) go to the fixup calls.
CALLS_K = [49, 34, 19, 9, 4, 2]
KMAX = len(CALLS_K)
FIX_CALLS = 2              # fixup slot-calls (each needs 2 gathers)
N_MAIN = sum(CALLS_K)      # 123 main (tgt-gather) calls
N_SLOT_CALLS = N_MAIN + FIX_CALLS          # 127 slot-calls
N_GATHER = N_MAIN + 2 * FIX_CALLS          # 131 gather instructions
ICOLS = CALL // 16         # 56 idx columns per call (wrapped layout)

F32 = mybir.dt.float32
F16 = mybir.dt.float16
I16 = mybir.dt.int16

_PROGRAM_CACHE = {}


def _dma_gather_small(g, out_ap, in_ap, idxs_ap, num_idxs, elem_size,
                      elem_step, queue_num):
    """nc.gpsimd.dma_gather, non-transpose, without the 256B elem_size
    assert (which is a transpose-mode restriction; HW decode only
    requires the row stride to be a 256B multiple)."""
    g._assert_queue_num(queue_num)
    assert idxs_ap.dtype == mybir.dt.int16
    assert in_ap.dtype == out_ap.dtype
    assert ap_utils.ap_is_contiguous(in_ap.ap[1:])
    assert ap_utils.ap_is_contiguous(out_ap.ap[1:])
    assert ap_utils.ap_is_contiguous(idxs_ap.ap[1:])
    assert in_ap.ap[-1][1] == out_ap.ap[-1][1] == elem_size
    assert out_ap.ap[0][1] * out_ap.ap[1][1] == CALL
    assert in_ap.ap[0][0] == elem_step
    stride_bytes = elem_step * mybir.dt.size(in_ap.dtype)
    stride_bytes_256 = stride_bytes // 256
    assert stride_bytes_256 * 256 == stride_bytes and stride_bytes_256 < 256
    _in_ap = g.lower_ap_dma(in_ap, for_custom_bir_dma=True)
    _idxs_ap = g.lower_ap(idxs_ap)
    _out_ap = g.lower_ap(out_ap)
    return g.add_instruction(
        mybir.InstDMAGatherAnt(
            name=g.bass.get_next_instruction_name(),
            ins=[*_in_ap, _idxs_ap,
                 g.lower_val_access(g.to_reg(num_idxs))],
            outs=[_out_ap],
            transpose=False,
            num_idxs=num_idxs,
            elem_size=elem_size,
            stride_bytes_256=stride_bytes_256,
            gen_mode=0,
            single_packet=True,
            queue_num=queue_num,
        )
    )


def _build_program():
    nc = bacc.Bacc("TRN2", num_swdge_queues=4)

    xT_in = nc.dram_tensor("xT_in", [F_IN, NP], F16, kind="ExternalInput")
    a_in = nc.dram_tensor("a_in", [F_IN, 8], F16, kind="ExternalInput")
    idx_in = nc.dram_tensor("idx_in", [128, N_GATHER * ICOLS], I16,
                            kind="ExternalInput")
    out_d = nc.dram_tensor("out", [128, N_SLOT_CALLS * WPP * K], F16,
                           kind="ExternalOutput")
    p_plain = nc.dram_tensor("p_plain", [128, NCH * 8], F16, kind="Internal")
    ptab = nc.dram_tensor("ptab", [NP, 128], F16, kind="Internal")

    # segment -> (first main call index, ncalls)
    seg_base = []
    b = 0
    for k in range(KMAX):
        seg_base.append(b)
        b += CALLS_K[k]

    with tile.TileContext(nc) as tc:
        with (
            tc.tile_pool(name="const", bufs=1) as cpool,
            tc.tile_pool(name="ps", bufs=2, space="PSUM") as ppool,
            tc.tile_pool(name="seg", bufs=1) as spool,
        ):
            a_raw = cpool.tile([F_IN, 8], F16)
            nc.sync.dma_start(out=a_raw[:], in_=a_in[:])
            a_sb = cpool.tile([F_IN, 8], F16)
            nc.vector.tensor_copy(out=a_sb[:], in_=a_raw[:])
            idx = cpool.tile([128, N_GATHER * ICOLS], I16)
            nc.sync.dma_start(out=idx[:], in_=idx_in[:])
            # xT arrives per-supertile so matmuls/casts/respreads pipeline
            xt = cpool.tile([F_IN, NP], F16)
            xt_done = 0
            while xt_done < NCH:
                m = min(64, NCH - xt_done)
                nc.sync.dma_start(
                    out=xt[:, 128 * xt_done:128 * (xt_done + m)],
                    in_=xT_in[:, 128 * xt_done:128 * (xt_done + m)])
                xt_done += m

            # ---- Phase A: p = [x.a_src | x.a_tgt] per node ----
            # Per 64-chunk supertile: matmuls -> f16 cast -> (a) write to
            # p_plain (contiguous, 128 descs) and (b) respread straight
            # into ptab's 256B-strided rows. The respreads (50K 16B
            # descriptors total) pipeline under the remaining matmuls
            # instead of serializing before the gathers.
            stage = cpool.tile([128, NCH * 8], F16)
            done = 0
            while done < NCH:
                m = min(64, NCH - done)
                ps = ppool.tile([128, 8 * m], F32)
                for i in range(m):
                    c = done + i
                    nc.tensor.matmul(
                        out=ps[:, 8 * i:8 * i + 8],
                        lhsT=xt[:, 128 * c:128 * c + 128],
                        rhs=a_sb[:, 0:8],
                        start=True,
                        stop=True,
                    )
                sl = stage[:, 8 * done:8 * (done + m)]
                nc.scalar.copy(out=sl, in_=ps[:, 0:8 * m])
                nc.sync.dma_start(
                    out=bass.AP(p_plain, 8 * done, [[NCH * 8, 128], [1, 8 * m]]),
                    in_=sl)
                # ptab rows ñ = p*391 + done + c', c' < m
                nc.sync.dma_start(
                    out=bass.AP(ptab, 128 * done,
                                [[NCH * 128, 128], [128, m], [1, 8]]),
                    in_=sl.rearrange("p (c e) -> p c e", e=8))
                done += m

            # ---- segment tiles + src-side affine expansion ----
            ds_tiles, dt_tiles, ad_tiles, o_tiles = [], [], [], []
            for k in range(KMAX):
                ncal = CALLS_K[k]
                dst = spool.tile([128, ncal * WPP * 8], F16, tag=f"ds{k}")
                dtt = spool.tile([128, ncal * WPP * 8], F16, tag=f"dt{k}")
                adt = spool.tile([128, ncal * WPP * K], F16, tag=f"ad{k}")
                ott = spool.tile([128, ncal * WPP * K], F16, tag=f"o{k}")
                ds_tiles.append(dst)
                dt_tiles.append(dtt)
                ad_tiles.append(adt)
                o_tiles.append(ott)
                # src AP: (p: 7 slots = 56 elems, call: 895 slots = 7160
                # elems, run: 56 elems) over p_plain's flat [NP*8] f16
                src = bass.AP(p_plain, 0,
                              [[56, 128], [7160, ncal], [1, 56]])
                dsv = dst[:].rearrange("p (cl e) -> p cl e", e=56)
                nc.sync.dma_start(out=dsv, in_=src)
            # fixup tiles
            dsf = spool.tile([128, FIX_CALLS * WPP * 8], F16, tag="dsf")
            dtf = spool.tile([128, FIX_CALLS * WPP * 8], F16, tag="dtf")
            adf = spool.tile([128, FIX_CALLS * WPP * K], F16, tag="adf")
            of = spool.tile([128, FIX_CALLS * WPP * K], F16, tag="of")

            # ---- tgt-side (and fixup src) gathers ----
            tab_ap = ptab[BIAS:, 0:8]
            all_g = []

            def gather(dst_tile, call_local, gidx):
                o = dst_tile[:, call_local * 56:(call_local + 1) * 56]
                gi = _dma_gather_small(
                    nc.gpsimd,
                    out_ap=o.rearrange("p (o m) -> p o m", o=WPP),
                    in_ap=tab_ap,
                    idxs_ap=idx[:, gidx * ICOLS:(gidx + 1) * ICOLS],
                    num_idxs=CALL,
                    elem_size=8,
                    elem_step=128,
                    queue_num=len(all_g) % 4,
                )
                if all_g:
                    ns = InstructionNameOrderedSet()
                    ns.add(all_g[-1].ins.name)
                    gi.ins.add_nosync_dependencies_from(ns)
                all_g.append(gi)

            # fixup gathers FIRST so their combine chain overlaps the main
            # gathers instead of trailing the whole kernel
            gidx = N_MAIN
            for cl in range(FIX_CALLS):      # fixup src gathers
                gather(dsf, cl, gidx)
                gidx += 1
            for cl in range(FIX_CALLS):      # fixup tgt gathers
                gather(dtf, cl, gidx)
                gidx += 1
            gidx = 0
            for k in range(KMAX):
                for cl in range(CALLS_K[k]):
                    gather(dt_tiles[k], cl, gidx)
                    gidx += 1

            # ---- combine: relu(DS[..,0:4] + DT[..,4:8]) ----
            def combine(dst, dtt, ad, ot, ncal):
                n_sl = ncal * WPP
                v0 = dst[:].rearrange("p (s e) -> p s e", e=8)[:, :, 0:4]
                v1 = dtt[:].rearrange("p (s e) -> p s e", e=8)[:, :, 4:8]
                av = ad[:].rearrange("p (s e) -> p s e", e=4)
                nc.vector.tensor_add(out=av, in0=v0, in1=v1)
                nc.scalar.activation(
                    out=ot[:], in_=ad[:],
                    func=mybir.ActivationFunctionType.Relu)

            for k in range(KMAX):
                combine(ds_tiles[k], dt_tiles[k], ad_tiles[k], o_tiles[k],
                        CALLS_K[k])
            combine(dsf, dtf, adf, of, FIX_CALLS)

            # ---- output DMAs ----
            col = 0
            for k in range(KMAX):
                w = CALLS_K[k] * WPP * K
                nc.sync.dma_start(out=out_d[:, col:col + w],
                                  in_=o_tiles[k][:])
                col += w
            w = FIX_CALLS * WPP * K
            nc.sync.dma_start(out=out_d[:, col:col + w], in_=of[:])

    # pin each gather's SWDGE queue to its scheduled completion-sem lane
    from concourse.tile_sem_assignment import PROC_NAME_TO_IDX
    lane_of = {PROC_NAME_TO_IDX[f"DMASW{i}"]: i for i in range(8)}
    for blk in nc.main_func.blocks:
        for inst in blk.instructions:
            if isinstance(inst, mybir.InstDMAGatherAnt):
                lane = lane_of.get(inst.bass_scheduled_proc)
                if lane is not None:
                    inst.queue_num = lane % 4

    nc.compile()
    return nc


def get_program():
    if "nc" not in _PROGRAM_CACHE:
        _PROGRAM_CACHE["nc"] = _build_program()
    return _PROGRAM_CACHE["nc"]


def _wrap_idx(vals):
    """Wrap a length-CALL idx vector for SWDGE: pos j -> [16g + j%16,
    j//16], replicated across the 8 gpsimd cores."""
    w = vals.reshape(ICOLS, 16).T.astype(np.int16)
    return np.tile(w, (8, 1))


def prepare_core(s, t, x16, att16):
    """Host marshaling for one core: node ordering, slot assignment,
    gather indices, input tensors, and the slot->edge output map."""
    E_c = len(s)
    d = np.bincount(s, minlength=N_NODES)
    order = np.argsort(-d, kind="stable")          # ñ -> orig node id
    rank_of = np.empty(N_NODES, dtype=np.int64)
    rank_of[order] = np.arange(N_NODES)

    # xT: node with ñ-rank u -> column 128*(u%391) + u//391, so that
    # p_plain row ñ (= p*391 + c for stage partition p chunk c) == u
    xT = np.zeros((F_IN, NP), dtype=np.float16)
    uu = np.arange(N_NODES)
    cols = 128 * (uu % NCH) + (uu // NCH)
    xT[:, cols] = x16[order].T                     # x rows in ñ order

    # per-edge src rank k
    o = np.argsort(s, kind="stable")
    so = s[o]
    starts = np.searchsorted(so, so)               # first pos of value
    kk = np.empty(E_c, dtype=np.int64)
    kk[o] = np.arange(E_c) - starts
    u = rank_of[s]
    tv = rank_of[t]

    # slot assignment
    call_no = np.full(E_c, -1, dtype=np.int64)
    r_no = np.full(E_c, -1, dtype=np.int64)
    seg_base = np.cumsum([0] + CALLS_K[:-1])
    ok = kk < KMAX
    capn = np.array([CALLS_K[k] * USE for k in range(KMAX)])
    ok &= u < capn[np.clip(kk, 0, KMAX - 1)]
    call_no[ok] = seg_base[kk[ok]] + u[ok] // USE
    r_no[ok] = u[ok] % USE
    fix = np.where(~ok)[0]
    if len(fix) > FIX_CALLS * USE:
        raise RuntimeError(f"fixup overflow: {len(fix)}")
    fpos = np.arange(len(fix))
    call_no[fix] = N_MAIN + fpos // USE
    r_no[fix] = fpos % USE

    # gather position j = (r%7)*128 + r//7
    j_no = (r_no % WPP) * 128 + r_no // WPP

    # gather idx array [128, N_GATHER*ICOLS]
    idx_arr = np.zeros((128, N_GATHER * ICOLS), dtype=np.int16)
    fixe = np.where(call_no >= N_MAIN)[0]

    tgt_vals = np.zeros((N_SLOT_CALLS, CALL), dtype=np.int64)
    tgt_vals[call_no, j_no] = tv - BIAS
    src_vals = np.zeros((FIX_CALLS, CALL), dtype=np.int64)
    src_vals[call_no[fixe] - N_MAIN, j_no[fixe]] = u[fixe] - BIAS

    g = 0
    for ci in range(N_MAIN):
        idx_arr[:, g * ICOLS:(g + 1) * ICOLS] = _wrap_idx(tgt_vals[ci])
        g += 1
    for ci in range(FIX_CALLS):
        idx_arr[:, g * ICOLS:(g + 1) * ICOLS] = _wrap_idx(src_vals[ci])
        g += 1
    for ci in range(FIX_CALLS):
        idx_arr[:, g * ICOLS:(g + 1) * ICOLS] = _wrap_idx(
            tgt_vals[N_MAIN + ci])
        g += 1

    # attention pack [64, 8]
    a = np.empty((F_IN, 8), dtype=np.float16)
    a[:, :K] = att16[:, :F_IN].T
    a[:, K:] = att16[:, F_IN:].T

    in_map = {"xT_in": xT, "a_in": a, "idx_in": idx_arr}
    # output location per edge: out_d[r//7, call*28 + (r%7)*4 + k]
    out_row = r_no // WPP
    out_col = call_no * (WPP * K) + (r_no % WPP) * K
    return in_map, out_row, out_col


def prepare_passes(x, edge_index, att):
    x16 = np.asarray(x, dtype=np.float32).astype(np.float16)
    att16 = np.asarray(att, dtype=np.float32).astype(np.float16)
    ei = np.asarray(edge_index).astype(np.int64)
    E_c = N_EDGES // CORES
    in_maps, maps = [], []
    for c in range(CORES):
        sl = slice(c * E_c, (c + 1) * E_c)
        # x16 rows must be passed in ñ order: prepare_core handles the
        # permutation internally via rank_of -> pass orig-order x
        im, orow, ocol = prepare_core(ei[0, sl], ei[1, sl], x16, att16)
        in_maps.append(im)
        maps.append((orow, ocol))
    return in_maps, maps


TRACE = False
LAST_RESULTS = []


def kernel(x, edge_index, att):
    nc = get_program()
    in_maps, maps = prepare_passes(x, edge_index, att)
    LAST_RESULTS.clear()
    res = run_bass_kernel_spmd(
        nc, in_maps, core_ids=list(range(CORES)), trace=TRACE)
    LAST_RESULTS.append(res)
    E_c = N_EDGES // CORES
    out = np.empty((N_EDGES, K), dtype=np.float32)
    for c in range(CORES):
        o = np.asarray(res.results[c]["out"])    # [128, cols] f16
        orow, ocol = maps[c]
        vals = o[orow[:, None], ocol[:, None] + np.arange(K)[None, :]]
        out[c * E_c:(c + 1) * E_c] = vals.astype(np.float32)
    return out


# revision 28
# speedup vs baseline: 1.1833x; 1.0323x over previous
"""Trainium2 Bass kernel for nn_MultiHeadLiftLayer (GNN edge-signal lift).

Computes, for each edge e with endpoints (s, t):
    out[e, k] = relu( x[s] . a_src[k] + x[t] . a_tgt[k] ),  k = 0..3

Architecture (v5, "rank-major expansion + single-side gather"):

The previous (baseline) kernel gathered both endpoints' x rows per edge
via SWDGE dma_gather; its trace shows the bottleneck is the Q7
descriptor-generation loop on the Pool engine (~2.5us per 896-idx call,
strictly serial) -- NOT DMA bandwidth. This version removes the entire
src side from the Q7 path:

  - Phase A (on device): p[n] = [x[n].a_src | x[n].a_tgt]  (8 f16 =
    16B per node) is computed by 391 node-major PE matmuls (lhsT =
    xT 128-node chunk, rhs = the 64x8 attention pack), cast f16 by the
    ACT engine, and stored to HBM as `p_plain` (contiguous 16B rows in
    a host-chosen node order) plus a 256B-strided copy `ptab` for the
    gather (SWDGE non-transpose gather requires a 256B-multiple row
    stride; element itself is 16B -- the bass-level 256B elem_size
    assert only applies to transpose mode, so we emit the instruction
    directly).
  - Node order ñ = per-core src-degree descending. Edge e is assigned
    slot (k = rank of e within its src node, u = ñ(s_e)). Slots are
    laid out rank-major: segment k holds nodes u < n_k (a PREFIX of ñ
    since ñ is degree-sorted). The src-side values for segment k are
    therefore a contiguous prefix of p_plain -- ONE affine 3-dim DMA
    per segment (no per-edge work at all).
  - The tgt side is the only per-edge gather: 16B rows from ptab via
    non-transpose dma_gather (idx j lands at partition j%128, word
    j//128 -- verified on HW). int16 indices cover all 50048 rows via a
    signed bias: the table AP is based at row 32768 and indices are
    ñ(t)-32768 (negative indices verified to address below the base on
    HW; CoreSim has an over-strict >=-1 assert, so sim mode is not
    supported for this kernel).
  - Calls carry 896 positions; position j=895 (slot r=895) is a
    structural pad so the trailing index of every call is >= 0 (the Q7
    ucode trims trailing negatives). A call covers 895 slots; slot r
    sits at (partition r//7, 16B-word r%7); gather position j =
    (r%7)*128 + r//7.
  - Combine: out = relu(DS[.., 0:4] + DT[.., 4:8]) with full
    128-partition parallelism (DVE add + ACT relu), f16, then one DMA
    per segment to HBM.
  - Edges that overflow the fixed per-segment capacities (src-rank >= 6,
    ~300 per core, or a segment fuller than mean+~8 sigma) go to 2
    fixup calls where BOTH endpoints are gathered; the fixup gathers are
    scheduled first so their combine chain hides under the main calls.
  - Gather instructions carry no explicit queue-spacing deps: the Q7
    ucode's own descriptor-ring await_space handles backpressure, so
    desc-gen runs at the serial Q7 floor (~2.0us/call) while the SDMA
    engines drain concurrently.

Measured: 409us vs 654us baseline (same rel err 5.7e-04). Remaining
profile: ~140us head (xT in + phase A + the 50K-descriptor respread --
fabric-bound, must complete before any gather), ~250us tgt-gather window
(Q7 desc-gen floor), ~20us tail.
"""

import numpy as np

import concourse.ap_utils as ap_utils
import concourse.bacc as bacc
import concourse.bass as bass
import concourse.mybir as mybir
import concourse.tile as tile
from concourse.bass_utils import run_bass_kernel_spmd
from concourse.instruction_name_ordered_set import InstructionNameOrderedSet

# ---- problem constants (hardcoded per contract) ----
N_NODES = 50000
N_EDGES = 800000
F_IN = 64
K = 4
CORES = 8

NP = 50176                 # padded node count = 128 * 392 (392 even:
                           # ñ-consecutive node PAIRS stay in-partition)
NCH = 392                  # node chunks of 128 (phase A matmuls)
QPP = NCH // 2             # node pairs per partition (196)
NPAIR = NP // 2            # ptab2 rows (25088, fits int16 unsigned-ish)
CALL = 896                 # gather positions per call
USE = 895                  # usable slots per call (pos 895 = pad)
WPP = 7                    # words per partition per call (896/128)

# fixed per-segment call capacities, k = 0..5 (per-core degrees are
# ~Poisson(2); capacities are mean + >=6.6 sigma). Edges with src-rank
# >= 6 (~300 per core) go to the fixup calls.
CALLS_K = [49, 34, 19, 9, 4, 2]
KMAX = len(CALLS_K)
FIX_CALLS = 2              # fixup slot-calls (each needs 2 gathers)
N_MAIN = sum(CALLS_K)      # 123 main (tgt-gather) calls
N_SLOT_CALLS = N_MAIN + FIX_CALLS          # 127 slot-calls
N_GATHER = N_MAIN + 2 * FIX_CALLS          # 131 gather instructions
ICOLS = CALL // 16         # 56 idx columns per call (wrapped layout)

F32 = mybir.dt.float32
F16 = mybir.dt.float16
I16 = mybir.dt.int16

_PROGRAM_CACHE = {}


def _dma_gather_small(g, out_ap, in_ap, idxs_ap, num_idxs, elem_size,
                      elem_step, queue_num):
    """nc.gpsimd.dma_gather, non-transpose, without the 256B elem_size
    assert (which is a transpose-mode restriction; HW decode only
    requires the row stride to be a 256B multiple)."""
    g._assert_queue_num(queue_num)
    assert idxs_ap.dtype == mybir.dt.int16
    assert in_ap.dtype == out_ap.dtype
    assert ap_utils.ap_is_contiguous(in_ap.ap[1:])
    assert ap_utils.ap_is_contiguous(out_ap.ap[1:])
    assert ap_utils.ap_is_contiguous(idxs_ap.ap[1:])
    assert in_ap.ap[-1][1] == out_ap.ap[-1][1] == elem_size
    assert out_ap.ap[0][1] * out_ap.ap[1][1] == CALL
    assert in_ap.ap[0][0] == elem_step
    stride_bytes = elem_step * mybir.dt.size(in_ap.dtype)
    stride_bytes_256 = stride_bytes // 256
    assert stride_bytes_256 * 256 == stride_bytes and stride_bytes_256 < 256
    _in_ap = g.lower_ap_dma(in_ap, for_custom_bir_dma=True)
    _idxs_ap = g.lower_ap(idxs_ap)
    _out_ap = g.lower_ap(out_ap)
    return g.add_instruction(
        mybir.InstDMAGatherAnt(
            name=g.bass.get_next_instruction_name(),
            ins=[*_in_ap, _idxs_ap,
                 g.lower_val_access(g.to_reg(num_idxs))],
            outs=[_out_ap],
            transpose=False,
            num_idxs=num_idxs,
            elem_size=elem_size,
            stride_bytes_256=stride_bytes_256,
            gen_mode=0,
            single_packet=True,
            queue_num=queue_num,
        )
    )


def _build_program():
    nc = bacc.Bacc("TRN2", num_swdge_queues=4)

    xT_in = nc.dram_tensor("xT_in", [F_IN, NP], F16, kind="ExternalInput")
    a_in = nc.dram_tensor("a_in", [F_IN, 8], F16, kind="ExternalInput")
    idx_in = nc.dram_tensor("idx_in", [128, N_GATHER * ICOLS], I16,
                            kind="ExternalInput")
    # parity masks (f16 0/1): tgt parity for every slot-call, then src
    # parity for the fixup calls; layout mirrors out_d's columns
    mask_in = nc.dram_tensor(
        "mask_in", [128, (N_SLOT_CALLS + FIX_CALLS) * WPP * K],
        mybir.dt.uint8, kind="ExternalInput")
    out_d = nc.dram_tensor("out", [128, N_SLOT_CALLS * WPP * K], F16,
                           kind="ExternalOutput")
    p_plain = nc.dram_tensor("p_plain", [128, NCH * 8], F16, kind="Internal")
    # paired table: row q holds nodes (2 per row within a partition's ñ
    # range): 32B payload at 256B stride -> half the respread descriptors
    ptab = nc.dram_tensor("ptab", [NPAIR, 128], F16, kind="Internal")

    # segment -> (first main call index, ncalls)
    seg_base = []
    b = 0
    for k in range(KMAX):
        seg_base.append(b)
        b += CALLS_K[k]

    with tile.TileContext(nc) as tc:
        with (
            tc.tile_pool(name="const", bufs=1) as cpool,
            tc.tile_pool(name="ps", bufs=2, space="PSUM") as ppool,
            tc.tile_pool(name="seg", bufs=1) as spool,
        ):
            a_raw = cpool.tile([F_IN, 8], F16)
            nc.sync.dma_start(out=a_raw[:], in_=a_in[:])
            a_sb = cpool.tile([F_IN, 8], F16)
            nc.vector.tensor_copy(out=a_sb[:], in_=a_raw[:])
            idx = cpool.tile([128, N_GATHER * ICOLS], I16)
            nc.sync.dma_start(out=idx[:], in_=idx_in[:])
            mtile = cpool.tile([128, (N_SLOT_CALLS + FIX_CALLS) * WPP * K],
                               mybir.dt.uint8)
            nc.sync.dma_start(out=mtile[:], in_=mask_in[:])
            # xT arrives per-supertile so matmuls/casts/respreads pipeline
            xt = cpool.tile([F_IN, NP], F16)
            xt_done = 0
            while xt_done < NCH:
                m = min(64, NCH - xt_done)
                nc.sync.dma_start(
                    out=xt[:, 128 * xt_done:128 * (xt_done + m)],
                    in_=xT_in[:, 128 * xt_done:128 * (xt_done + m)])
                xt_done += m

            # ---- Phase A: p = [x.a_src | x.a_tgt] per node ----
            # Per 64-chunk supertile: matmuls -> f16 cast -> (a) write to
            # p_plain (contiguous, 128 descs) and (b) respread straight
            # into ptab's 256B-strided rows. The respreads (50K 16B
            # descriptors total) pipeline under the remaining matmuls
            # instead of serializing before the gathers.
            stage = cpool.tile([128, NCH * 8], F16)
            done = 0
            while done < NCH:
                m = min(64, NCH - done)
                ps = ppool.tile([128, 8 * m], F32)
                for i in range(m):
                    c = done + i
                    nc.tensor.matmul(
                        out=ps[:, 8 * i:8 * i + 8],
                        lhsT=xt[:, 128 * c:128 * c + 128],
                        rhs=a_sb[:, 0:8],
                        start=True,
                        stop=True,
                    )
                sl = stage[:, 8 * done:8 * (done + m)]
                nc.scalar.copy(out=sl, in_=ps[:, 0:8 * m])
                nc.sync.dma_start(
                    out=bass.AP(p_plain, 8 * done, [[NCH * 8, 128], [1, 8 * m]]),
                    in_=sl)
                # ptab pair-rows q = p*196 + (done+c)/2, 32B payload each
                nc.sync.dma_start(
                    out=bass.AP(ptab, 128 * (done // 2),
                                [[QPP * 128, 128], [128, m // 2], [1, 16]]),
                    in_=sl.rearrange("p (q e) -> p q e", e=16))
                done += m

            # ---- segment tiles (DT holds 32B pair-rows per slot) ----
            ds_tiles, dt_tiles, sl_tiles, ad_tiles, o_tiles = \
                [], [], [], [], []
            for k in range(KMAX):
                ncal = CALLS_K[k]
                dst = spool.tile([128, ncal * WPP * 8], F16, tag=f"ds{k}")
                dtt = spool.tile([128, ncal * WPP * 16], F16, tag=f"dt{k}")
                slt = spool.tile([128, ncal * WPP * K], F16, tag=f"sl{k}")
                adt = spool.tile([128, ncal * WPP * K], F16, tag=f"ad{k}")
                ott = spool.tile([128, ncal * WPP * K], F16, tag=f"o{k}")
                ds_tiles.append(dst)
                dt_tiles.append(dtt)
                sl_tiles.append(slt)
                ad_tiles.append(adt)
                o_tiles.append(ott)
            # fixup tiles
            dsf = spool.tile([128, FIX_CALLS * WPP * 16], F16, tag="dsf")
            dtf = spool.tile([128, FIX_CALLS * WPP * 16], F16, tag="dtf")
            slsf = spool.tile([128, FIX_CALLS * WPP * K], F16, tag="slsf")
            sltf = spool.tile([128, FIX_CALLS * WPP * K], F16, tag="sltf")
            adf = spool.tile([128, FIX_CALLS * WPP * K], F16, tag="adf")
            of = spool.tile([128, FIX_CALLS * WPP * K], F16, tag="of")

            # ---- tgt-side (and fixup src) gathers ----
            tab_ap = ptab[:, 0:16]
            all_g = []

            def gather(dst_tile, call_local, gidx):
                o = dst_tile[:, call_local * 112:(call_local + 1) * 112]
                gi = _dma_gather_small(
                    nc.gpsimd,
                    out_ap=o.rearrange("p (o m) -> p o m", o=WPP),
                    in_ap=tab_ap,
                    idxs_ap=idx[:, gidx * ICOLS:(gidx + 1) * ICOLS],
                    num_idxs=CALL,
                    elem_size=16,
                    elem_step=128,
                    queue_num=len(all_g) % 4,
                )
                if all_g:
                    ns = InstructionNameOrderedSet()
                    ns.add(all_g[-1].ins.name)
                    gi.ins.add_nosync_dependencies_from(ns)
                all_g.append(gi)

            # fixup gathers FIRST so their combine chain overlaps the main
            # gathers instead of trailing the whole kernel
            gidx = N_MAIN
            for cl in range(FIX_CALLS):      # fixup src gathers
                gather(dsf, cl, gidx)
                gidx += 1
            for cl in range(FIX_CALLS):      # fixup tgt gathers
                gather(dtf, cl, gidx)
                gidx += 1
            gidx = 0
            for k in range(KMAX):
                for cl in range(CALLS_K[k]):
                    gather(dt_tiles[k], cl, gidx)
                    gidx += 1

            # ---- src-side affine expansion (emitted after the gathers
            # so its fabric time drains during the gather window, not
            # before it) ----
            for k in range(KMAX):
                ncal = CALLS_K[k]
                # src AP: (p: 7 slots = 56 elems, call: 895 slots = 7160
                # elems, run: 56 elems) over p_plain's flat [NP*8] f16
                src = bass.AP(p_plain, 0,
                              [[56, 128], [7160, ncal], [1, 56]])
                dsv = ds_tiles[k][:].rearrange("p (cl e) -> p cl e", e=56)
                nc.sync.dma_start(out=dsv, in_=src)

            # ---- combine: relu(DS.ps + select(parity, DT.odd, DT.even))
            def sel_half(dtt, slt, ncal, mask_col, base_off):
                n_sl = ncal * WPP
                d16 = dtt[:].rearrange("p (s e) -> p s e", e=16)
                mv = mtile[:, mask_col:mask_col + n_sl * K].rearrange(
                    "p (s e) -> p s e", e=4)
                sv = slt[:].rearrange("p (s e) -> p s e", e=4)
                nc.vector.select(out=sv, mask=mv,
                                 on_true=d16[:, :, base_off + 8:
                                             base_off + 12],
                                 on_false=d16[:, :, base_off:base_off + 4])

            for k, kb in enumerate(seg_base):
                ncal = CALLS_K[k]
                sel_half(dt_tiles[k], sl_tiles[k], ncal, kb * WPP * K, 4)
                v0 = ds_tiles[k][:].rearrange(
                    "p (s e) -> p s e", e=8)[:, :, 0:4]
                av = ad_tiles[k][:].rearrange("p (s e) -> p s e", e=4)
                sv = sl_tiles[k][:].rearrange("p (s e) -> p s e", e=4)
                nc.vector.tensor_add(out=av, in0=v0, in1=sv)
                nc.scalar.activation(
                    out=o_tiles[k][:], in_=ad_tiles[k][:],
                    func=mybir.ActivationFunctionType.Relu)
            # fixup: both sides gathered pair-rows; select each
            sel_half(dtf, sltf, FIX_CALLS, N_MAIN * WPP * K, 4)
            sel_half(dsf, slsf, FIX_CALLS, N_SLOT_CALLS * WPP * K, 0)
            nc.vector.tensor_add(out=adf[:], in0=slsf[:], in1=sltf[:])
            nc.scalar.activation(out=of[:], in_=adf[:],
                                 func=mybir.ActivationFunctionType.Relu)

            # ---- output DMAs ----
            col = 0
            for k in range(KMAX):
                w = CALLS_K[k] * WPP * K
                nc.sync.dma_start(out=out_d[:, col:col + w],
                                  in_=o_tiles[k][:])
                col += w
            w = FIX_CALLS * WPP * K
            nc.sync.dma_start(out=out_d[:, col:col + w], in_=of[:])

    # pin each gather's SWDGE queue to its scheduled completion-sem lane
    from concourse.tile_sem_assignment import PROC_NAME_TO_IDX
    lane_of = {PROC_NAME_TO_IDX[f"DMASW{i}"]: i for i in range(8)}
    for blk in nc.main_func.blocks:
        for inst in blk.instructions:
            if isinstance(inst, mybir.InstDMAGatherAnt):
                lane = lane_of.get(inst.bass_scheduled_proc)
                if lane is not None:
                    inst.queue_num = lane % 4

    nc.compile()
    return nc


def get_program():
    if "nc" not in _PROGRAM_CACHE:
        _PROGRAM_CACHE["nc"] = _build_program()
    return _PROGRAM_CACHE["nc"]


def _wrap_idx(vals):
    """Wrap a length-CALL idx vector for SWDGE: pos j -> [16g + j%16,
    j//16], replicated across the 8 gpsimd cores."""
    w = vals.reshape(ICOLS, 16).T.astype(np.int16)
    return np.tile(w, (8, 1))


def prepare_core(s, t, x16, att16):
    """Host marshaling for one core: node ordering, slot assignment,
    gather indices, input tensors, and the slot->edge output map."""
    E_c = len(s)
    d = np.bincount(s, minlength=N_NODES)
    order = np.argsort(-d, kind="stable")          # ñ -> orig node id
    rank_of = np.empty(N_NODES, dtype=np.int64)
    rank_of[order] = np.arange(N_NODES)

    # xT: node with ñ-rank u -> column 128*(u%392) + u//392, so that
    # p_plain row ñ (= p*392 + c for stage partition p chunk c) == u
    xT = np.zeros((F_IN, NP), dtype=np.float16)
    uu = np.arange(N_NODES)
    cols = 128 * (uu % NCH) + (uu // NCH)
    xT[:, cols] = x16[order].T                     # x rows in ñ order

    # per-edge src rank k
    o = np.argsort(s, kind="stable")
    so = s[o]
    starts = np.searchsorted(so, so)               # first pos of value
    kk = np.empty(E_c, dtype=np.int64)
    kk[o] = np.arange(E_c) - starts
    u = rank_of[s]
    tv = rank_of[t]

    # slot assignment
    call_no = np.full(E_c, -1, dtype=np.int64)
    r_no = np.full(E_c, -1, dtype=np.int64)
    seg_base = np.cumsum([0] + CALLS_K[:-1])
    ok = kk < KMAX
    capn = np.array([CALLS_K[k] * USE for k in range(KMAX)])
    ok &= u < capn[np.clip(kk, 0, KMAX - 1)]
    call_no[ok] = seg_base[kk[ok]] + u[ok] // USE
    r_no[ok] = u[ok] % USE
    fix = np.where(~ok)[0]
    if len(fix) > FIX_CALLS * USE:
        raise RuntimeError(f"fixup overflow: {len(fix)}")
    fpos = np.arange(len(fix))
    call_no[fix] = N_MAIN + fpos // USE
    r_no[fix] = fpos % USE

    # gather position j = (r%7)*128 + r//7
    j_no = (r_no % WPP) * 128 + r_no // WPP

    # gather idx array [128, N_GATHER*ICOLS]: idx = ptab pair-row of the
    # node's ñ-rank: (ñ//392)*196 + (ñ%392)//2; parity bit = ñ%2
    idx_arr = np.zeros((128, N_GATHER * ICOLS), dtype=np.int16)
    fixe = np.where(call_no >= N_MAIN)[0]

    def pair_row(r):
        return (r // NCH) * QPP + (r % NCH) // 2

    tgt_vals = np.zeros((N_SLOT_CALLS, CALL), dtype=np.int64)
    tgt_vals[call_no, j_no] = pair_row(tv)
    src_vals = np.zeros((FIX_CALLS, CALL), dtype=np.int64)
    src_vals[call_no[fixe] - N_MAIN, j_no[fixe]] = pair_row(u[fixe])

    g = 0
    for ci in range(N_MAIN):
        idx_arr[:, g * ICOLS:(g + 1) * ICOLS] = _wrap_idx(tgt_vals[ci])
        g += 1
    for ci in range(FIX_CALLS):
        idx_arr[:, g * ICOLS:(g + 1) * ICOLS] = _wrap_idx(src_vals[ci])
        g += 1
    for ci in range(FIX_CALLS):
        idx_arr[:, g * ICOLS:(g + 1) * ICOLS] = _wrap_idx(
            tgt_vals[N_MAIN + ci])
        g += 1

    # attention pack [64, 8]
    a = np.empty((F_IN, 8), dtype=np.float16)
    a[:, :K] = att16[:, :F_IN].T
    a[:, K:] = att16[:, F_IN:].T

    # output location per edge: out_d[r//7, call*28 + (r%7)*4 + k]
    out_row = r_no // WPP
    out_col = call_no * (WPP * K) + (r_no % WPP) * K

    # parity masks, laid out like out_d columns (plus fixup-src block)
    mask = np.zeros((128, (N_SLOT_CALLS + FIX_CALLS) * WPP * K),
                    dtype=np.uint8)
    k4 = np.arange(K)[None, :]
    mask[out_row[:, None], out_col[:, None] + k4] = \
        (tv % 2).astype(np.uint8)[:, None]
    mask[out_row[fixe][:, None],
         out_col[fixe][:, None] + 2 * WPP * K + k4] = \
        (u[fixe] % 2).astype(np.uint8)[:, None]

    in_map = {"xT_in": xT, "a_in": a, "idx_in": idx_arr, "mask_in": mask}
    return in_map, out_row, out_col


def prepare_passes(x, edge_index, att):
    x16 = np.asarray(x, dtype=np.float32).astype(np.float16)
    att16 = np.asarray(att, dtype=np.float32).astype(np.float16)
    ei = np.asarray(edge_index).astype(np.int64)
    E_c = N_EDGES // CORES
    in_maps, maps = [], []
    for c in range(CORES):
        sl = slice(c * E_c, (c + 1) * E_c)
        # x16 rows must be passed in ñ order: prepare_core handles the
        # permutation internally via rank_of -> pass orig-order x
        im, orow, ocol = prepare_core(ei[0, sl], ei[1, sl], x16, att16)
        in_maps.append(im)
        maps.append((orow, ocol))
    return in_maps, maps


TRACE = False
LAST_RESULTS = []


def kernel(x, edge_index, att):
    nc = get_program()
    in_maps, maps = prepare_passes(x, edge_index, att)
    LAST_RESULTS.clear()
    res = run_bass_kernel_spmd(
        nc, in_maps, core_ids=list(range(CORES)), trace=TRACE)
    LAST_RESULTS.append(res)
    E_c = N_EDGES // CORES
    out = np.empty((N_EDGES, K), dtype=np.float32)
    for c in range(CORES):
        o = np.asarray(res.results[c]["out"])    # [128, cols] f16
        orow, ocol = maps[c]
        vals = o[orow[:, None], ocol[:, None] + np.arange(K)[None, :]]
        out[c * E_c:(c + 1) * E_c] = vals.astype(np.float32)
    return out


# revision 30
# speedup vs baseline: 1.2443x; 1.0515x over previous
"""Trainium2 Bass kernel for nn_MultiHeadLiftLayer (GNN edge-signal lift).

Computes, for each edge e with endpoints (s, t):
    out[e, k] = relu( x[s] . a_src[k] + x[t] . a_tgt[k] ),  k = 0..3

Architecture (v5, "rank-major expansion + single-side gather"):

The previous (baseline) kernel gathered both endpoints' x rows per edge
via SWDGE dma_gather; its trace shows the bottleneck is the Q7
descriptor-generation loop on the Pool engine (~2.5us per 896-idx call,
strictly serial) -- NOT DMA bandwidth. This version removes the entire
src side from the Q7 path:

  - Phase A (on device): p[n] = [x[n].a_src | x[n].a_tgt]  (8 f16 =
    16B per node) is computed by 391 node-major PE matmuls (lhsT =
    xT 128-node chunk, rhs = the 64x8 attention pack), cast f16 by the
    ACT engine, and stored to HBM as `p_plain` (contiguous 16B rows in
    a host-chosen node order) plus a 256B-strided copy `ptab` for the
    gather (SWDGE non-transpose gather requires a 256B-multiple row
    stride; element itself is 16B -- the bass-level 256B elem_size
    assert only applies to transpose mode, so we emit the instruction
    directly).
  - Node order ñ = per-core src-degree descending. Edge e is assigned
    slot (k = rank of e within its src node, u = ñ(s_e)). Slots are
    laid out rank-major: segment k holds nodes u < n_k (a PREFIX of ñ
    since ñ is degree-sorted). The src-side values for segment k are
    therefore a contiguous prefix of p_plain -- ONE affine 3-dim DMA
    per segment (no per-edge work at all).
  - The tgt side is the only per-edge gather: 16B rows from ptab via
    non-transpose dma_gather (idx j lands at partition j%128, word
    j//128 -- verified on HW). int16 indices cover all 50048 rows via a
    signed bias: the table AP is based at row 32768 and indices are
    ñ(t)-32768 (negative indices verified to address below the base on
    HW; CoreSim has an over-strict >=-1 assert, so sim mode is not
    supported for this kernel).
  - Calls carry 896 positions; position j=895 (slot r=895) is a
    structural pad so the trailing index of every call is >= 0 (the Q7
    ucode trims trailing negatives). A call covers 895 slots; slot r
    sits at (partition r//7, 16B-word r%7); gather position j =
    (r%7)*128 + r//7.
  - Combine: out = relu(DS[.., 0:4] + DT[.., 4:8]) with full
    128-partition parallelism (DVE add + ACT relu), f16, then one DMA
    per segment to HBM.
  - Edges that overflow the fixed per-segment capacities (src-rank >= 6,
    ~300 per core, or a segment fuller than mean+~8 sigma) go to 2
    fixup calls where BOTH endpoints are gathered; the fixup gathers are
    scheduled first so their combine chain hides under the main calls.
  - Gather instructions carry no explicit queue-spacing deps: the Q7
    ucode's own descriptor-ring await_space handles backpressure, so
    desc-gen runs at the serial Q7 floor (~2.0us/call) while the SDMA
    engines drain concurrently.

Measured: 409us vs 654us baseline (same rel err 5.7e-04). Remaining
profile: ~140us head (xT in + phase A + the 50K-descriptor respread --
fabric-bound, must complete before any gather), ~250us tgt-gather window
(Q7 desc-gen floor), ~20us tail.
"""

import numpy as np

import concourse.ap_utils as ap_utils
import concourse.bacc as bacc
import concourse.bass as bass
import concourse.mybir as mybir
import concourse.tile as tile
from concourse.bass_utils import run_bass_kernel_spmd
from concourse.instruction_name_ordered_set import InstructionNameOrderedSet

# ---- problem constants (hardcoded per contract) ----
N_NODES = 50000
N_EDGES = 800000
F_IN = 64
K = 4
CORES = 8

NP = 50176                 # padded node count = 128 * 392 (392 even:
                           # ñ-consecutive node PAIRS stay in-partition)
NCH = 392                  # node chunks of 128 (phase A matmuls)
QPP = NCH // 2             # node pairs per partition (196)
NPAIR = NP // 2            # ptab2 rows (25088, fits int16 unsigned-ish)
CALL = 896                 # gather positions per call
USE = 895                  # usable slots per call (pos 895 = pad)
WPP = 7                    # words per partition per call (896/128)

# fixed per-segment call capacities, k = 0..5 (per-core degrees are
# ~Poisson(2); capacities are mean + >=6.6 sigma). Edges with src-rank
# >= 6 (~300 per core) go to the fixup calls.
CALLS_K = [49, 34, 19, 9, 4, 2]
KMAX = len(CALLS_K)
FIX_CALLS = 2              # fixup slot-calls (each needs 2 gathers)
N_MAIN = sum(CALLS_K)      # 123 main (tgt-gather) calls
N_SLOT_CALLS = N_MAIN + FIX_CALLS          # 127 slot-calls
N_GATHER = N_MAIN + 2 * FIX_CALLS          # 131 gather instructions
ICOLS = CALL // 16         # 56 idx columns per call (wrapped layout)

F32 = mybir.dt.float32
F16 = mybir.dt.float16
I16 = mybir.dt.int16

_PROGRAM_CACHE = {}


def _dma_gather_small(g, out_ap, in_ap, idxs_ap, num_idxs, elem_size,
                      elem_step, queue_num):
    """nc.gpsimd.dma_gather, non-transpose, without the 256B elem_size
    assert (which is a transpose-mode restriction; HW decode only
    requires the row stride to be a 256B multiple)."""
    g._assert_queue_num(queue_num)
    assert idxs_ap.dtype == mybir.dt.int16
    assert in_ap.dtype == out_ap.dtype
    assert ap_utils.ap_is_contiguous(in_ap.ap[1:])
    assert ap_utils.ap_is_contiguous(out_ap.ap[1:])
    assert ap_utils.ap_is_contiguous(idxs_ap.ap[1:])
    assert in_ap.ap[-1][1] == out_ap.ap[-1][1] == elem_size
    assert out_ap.ap[0][1] * out_ap.ap[1][1] == CALL
    assert in_ap.ap[0][0] == elem_step
    stride_bytes = elem_step * mybir.dt.size(in_ap.dtype)
    stride_bytes_256 = stride_bytes // 256
    assert stride_bytes_256 * 256 == stride_bytes and stride_bytes_256 < 256
    _in_ap = g.lower_ap_dma(in_ap, for_custom_bir_dma=True)
    _idxs_ap = g.lower_ap(idxs_ap)
    _out_ap = g.lower_ap(out_ap)
    return g.add_instruction(
        mybir.InstDMAGatherAnt(
            name=g.bass.get_next_instruction_name(),
            ins=[*_in_ap, _idxs_ap,
                 g.lower_val_access(g.to_reg(num_idxs))],
            outs=[_out_ap],
            transpose=False,
            num_idxs=num_idxs,
            elem_size=elem_size,
            stride_bytes_256=stride_bytes_256,
            gen_mode=0,
            single_packet=True,
            queue_num=queue_num,
        )
    )


def _build_program():
    nc = bacc.Bacc("TRN2", num_swdge_queues=4)

    xT_in = nc.dram_tensor("xT_in", [F_IN, NP], F16, kind="ExternalInput")
    a_in = nc.dram_tensor("a_in", [F_IN, 8], F16, kind="ExternalInput")
    idx_in = nc.dram_tensor("idx_in", [128, N_GATHER * ICOLS], I16,
                            kind="ExternalInput")
    # parity masks (f16 0/1): tgt parity for every slot-call, then src
    # parity for the fixup calls; layout mirrors out_d's columns
    mask_in = nc.dram_tensor(
        "mask_in", [128, (N_SLOT_CALLS + FIX_CALLS) * WPP * K],
        mybir.dt.uint8, kind="ExternalInput")
    out_d = nc.dram_tensor("out", [128, N_SLOT_CALLS * WPP * K], F16,
                           kind="ExternalOutput")
    p_plain = nc.dram_tensor("p_plain", [128, NCH * 8], F16, kind="Internal")
    # paired table: row q holds nodes (2 per row within a partition's ñ
    # range): 32B payload at 256B stride -> half the respread descriptors
    ptab = nc.dram_tensor("ptab", [NPAIR, 128], F16, kind="Internal")

    # segment -> (first main call index, ncalls)
    seg_base = []
    b = 0
    for k in range(KMAX):
        seg_base.append(b)
        b += CALLS_K[k]

    with tile.TileContext(nc) as tc:
        with (
            tc.tile_pool(name="const", bufs=1) as cpool,
            tc.tile_pool(name="ps", bufs=2, space="PSUM") as ppool,
            tc.tile_pool(name="seg", bufs=1) as spool,
        ):
            a_raw = cpool.tile([F_IN, 8], F16)
            nc.sync.dma_start(out=a_raw[:], in_=a_in[:])
            a_sb = cpool.tile([F_IN, 8], F16)
            nc.vector.tensor_copy(out=a_sb[:], in_=a_raw[:])
            idx = cpool.tile([128, N_GATHER * ICOLS], I16)
            nc.sync.dma_start(out=idx[:], in_=idx_in[:])
            mtile = cpool.tile([128, (N_SLOT_CALLS + FIX_CALLS) * WPP * K],
                               mybir.dt.uint8)
            nc.sync.dma_start(out=mtile[:], in_=mask_in[:])
            # xT arrives per-supertile so matmuls/casts/respreads pipeline
            xt = cpool.tile([F_IN, NP], F16)
            xt_done = 0
            while xt_done < NCH:
                m = min(64, NCH - xt_done)
                nc.sync.dma_start(
                    out=xt[:, 128 * xt_done:128 * (xt_done + m)],
                    in_=xT_in[:, 128 * xt_done:128 * (xt_done + m)])
                xt_done += m

            # ---- Phase A: p = [x.a_src | x.a_tgt] per node ----
            # Per 64-chunk supertile: matmuls -> f16 cast -> (a) write to
            # p_plain (contiguous, 128 descs) and (b) respread straight
            # into ptab's 256B-strided rows. The respreads (50K 16B
            # descriptors total) pipeline under the remaining matmuls
            # instead of serializing before the gathers.
            stage = cpool.tile([128, NCH * 8], F16)
            done = 0
            while done < NCH:
                m = min(64, NCH - done)
                ps = ppool.tile([128, 8 * m], F32)
                for i in range(m):
                    c = done + i
                    nc.tensor.matmul(
                        out=ps[:, 8 * i:8 * i + 8],
                        lhsT=xt[:, 128 * c:128 * c + 128],
                        rhs=a_sb[:, 0:8],
                        start=True,
                        stop=True,
                    )
                sl = stage[:, 8 * done:8 * (done + m)]
                nc.scalar.copy(out=sl, in_=ps[:, 0:8 * m])
                nc.sync.dma_start(
                    out=bass.AP(p_plain, 8 * done, [[NCH * 8, 128], [1, 8 * m]]),
                    in_=sl)
                # ptab pair-rows q = p*196 + (done+c)/2, 32B payload each
                nc.sync.dma_start(
                    out=bass.AP(ptab, 128 * (done // 2),
                                [[QPP * 128, 128], [128, m // 2], [1, 16]]),
                    in_=sl.rearrange("p (q e) -> p q e", e=16))
                done += m

            # ---- segment tiles (DT holds 32B pair-rows per slot) ----
            ds_tiles, dt_tiles, ad_tiles, o_tiles = [], [], [], []
            for k in range(KMAX):
                ncal = CALLS_K[k]
                dst = spool.tile([128, ncal * WPP * 8], F16, tag=f"ds{k}")
                dtt = spool.tile([128, ncal * WPP * 16], F16, tag=f"dt{k}")
                adt = spool.tile([128, ncal * WPP * K], F16, tag=f"ad{k}")
                ott = spool.tile([128, ncal * WPP * K], F16, tag=f"o{k}")
                ds_tiles.append(dst)
                dt_tiles.append(dtt)
                ad_tiles.append(adt)
                o_tiles.append(ott)
            # fixup tiles
            dsf = spool.tile([128, FIX_CALLS * WPP * 16], F16, tag="dsf")
            dtf = spool.tile([128, FIX_CALLS * WPP * 16], F16, tag="dtf")
            adf = spool.tile([128, FIX_CALLS * WPP * K], F16, tag="adf")
            of = spool.tile([128, FIX_CALLS * WPP * K], F16, tag="of")

            # ---- tgt-side (and fixup src) gathers ----
            tab_ap = ptab[:, 0:16]
            all_g = []

            def gather(dst_tile, call_local, gidx):
                o = dst_tile[:, call_local * 112:(call_local + 1) * 112]
                gi = _dma_gather_small(
                    nc.gpsimd,
                    out_ap=o.rearrange("p (o m) -> p o m", o=WPP),
                    in_ap=tab_ap,
                    idxs_ap=idx[:, gidx * ICOLS:(gidx + 1) * ICOLS],
                    num_idxs=CALL,
                    elem_size=16,
                    elem_step=128,
                    queue_num=len(all_g) % 4,
                )
                if all_g:
                    ns = InstructionNameOrderedSet()
                    ns.add(all_g[-1].ins.name)
                    gi.ins.add_nosync_dependencies_from(ns)
                all_g.append(gi)

            # fixup gathers FIRST so their combine chain overlaps the main
            # gathers instead of trailing the whole kernel
            gidx = N_MAIN
            for cl in range(FIX_CALLS):      # fixup src gathers
                gather(dsf, cl, gidx)
                gidx += 1
            for cl in range(FIX_CALLS):      # fixup tgt gathers
                gather(dtf, cl, gidx)
                gidx += 1
            gidx = 0
            for k in range(KMAX):
                for cl in range(CALLS_K[k]):
                    gather(dt_tiles[k], cl, gidx)
                    gidx += 1

            # ---- src-side affine expansion (emitted after the gathers
            # so its fabric time drains during the gather window, not
            # before it) ----
            for k in range(KMAX):
                ncal = CALLS_K[k]
                # src AP: (p: 7 slots = 56 elems, call: 895 slots = 7160
                # elems, run: 56 elems) over p_plain's flat [NP*8] f16
                src = bass.AP(p_plain, 0,
                              [[56, 128], [7160, ncal], [1, 56]])
                dsv = ds_tiles[k][:].rearrange("p (cl e) -> p cl e", e=56)
                nc.sync.dma_start(out=dsv, in_=src)

            # ---- combine: where parity, overwrite the even-node half
            # with the odd-node half IN PLACE (copy_predicated is cheap;
            # a separate select would pay a pathological strided
            # tensor_copy), then add + relu
            def sel_half(dtt, ncal, mask_col, base_off):
                n_sl = ncal * WPP
                d16 = dtt[:].rearrange("p (s e) -> p s e", e=16)
                mv = mtile[:, mask_col:mask_col + n_sl * K].rearrange(
                    "p (s e) -> p s e", e=4)
                nc.vector.copy_predicated(
                    out=d16[:, :, base_off:base_off + 4], mask=mv,
                    data=d16[:, :, base_off + 8:base_off + 12])
                return d16[:, :, base_off:base_off + 4]

            for k, kb in enumerate(seg_base):
                ncal = CALLS_K[k]
                sv = sel_half(dt_tiles[k], ncal, kb * WPP * K, 4)
                v0 = ds_tiles[k][:].rearrange(
                    "p (s e) -> p s e", e=8)[:, :, 0:4]
                av = ad_tiles[k][:].rearrange("p (s e) -> p s e", e=4)
                nc.vector.tensor_add(out=av, in0=v0, in1=sv)
                nc.scalar.activation(
                    out=o_tiles[k][:], in_=ad_tiles[k][:],
                    func=mybir.ActivationFunctionType.Relu)
            # fixup: both sides gathered pair-rows; select each
            svt = sel_half(dtf, FIX_CALLS, N_MAIN * WPP * K, 4)
            svs = sel_half(dsf, FIX_CALLS, N_SLOT_CALLS * WPP * K, 0)
            adfv = adf[:].rearrange("p (s e) -> p s e", e=4)
            nc.vector.tensor_add(out=adfv, in0=svs, in1=svt)
            nc.scalar.activation(out=of[:], in_=adf[:],
                                 func=mybir.ActivationFunctionType.Relu)

            # ---- output DMAs ----
            col = 0
            for k in range(KMAX):
                w = CALLS_K[k] * WPP * K
                nc.sync.dma_start(out=out_d[:, col:col + w],
                                  in_=o_tiles[k][:])
                col += w
            w = FIX_CALLS * WPP * K
            nc.sync.dma_start(out=out_d[:, col:col + w], in_=of[:])

    # pin each gather's SWDGE queue to its scheduled completion-sem lane
    from concourse.tile_sem_assignment import PROC_NAME_TO_IDX
    lane_of = {PROC_NAME_TO_IDX[f"DMASW{i}"]: i for i in range(8)}
    for blk in nc.main_func.blocks:
        for inst in blk.instructions:
            if isinstance(inst, mybir.InstDMAGatherAnt):
                lane = lane_of.get(inst.bass_scheduled_proc)
                if lane is not None:
                    inst.queue_num = lane % 4

    nc.compile()
    return nc


def get_program():
    if "nc" not in _PROGRAM_CACHE:
        _PROGRAM_CACHE["nc"] = _build_program()
    return _PROGRAM_CACHE["nc"]


def _wrap_idx(vals):
    """Wrap a length-CALL idx vector for SWDGE: pos j -> [16g + j%16,
    j//16], replicated across the 8 gpsimd cores."""
    w = vals.reshape(ICOLS, 16).T.astype(np.int16)
    return np.tile(w, (8, 1))


def prepare_core(s, t, x16, att16):
    """Host marshaling for one core: node ordering, slot assignment,
    gather indices, input tensors, and the slot->edge output map."""
    E_c = len(s)
    d = np.bincount(s, minlength=N_NODES)
    order = np.argsort(-d, kind="stable")          # ñ -> orig node id
    rank_of = np.empty(N_NODES, dtype=np.int64)
    rank_of[order] = np.arange(N_NODES)

    # xT: node with ñ-rank u -> column 128*(u%392) + u//392, so that
    # p_plain row ñ (= p*392 + c for stage partition p chunk c) == u
    xT = np.zeros((F_IN, NP), dtype=np.float16)
    uu = np.arange(N_NODES)
    cols = 128 * (uu % NCH) + (uu // NCH)
    xT[:, cols] = x16[order].T                     # x rows in ñ order

    # per-edge src rank k
    o = np.argsort(s, kind="stable")
    so = s[o]
    starts = np.searchsorted(so, so)               # first pos of value
    kk = np.empty(E_c, dtype=np.int64)
    kk[o] = np.arange(E_c) - starts
    u = rank_of[s]
    tv = rank_of[t]

    # slot assignment
    call_no = np.full(E_c, -1, dtype=np.int64)
    r_no = np.full(E_c, -1, dtype=np.int64)
    seg_base = np.cumsum([0] + CALLS_K[:-1])
    ok = kk < KMAX
    capn = np.array([CALLS_K[k] * USE for k in range(KMAX)])
    ok &= u < capn[np.clip(kk, 0, KMAX - 1)]
    call_no[ok] = seg_base[kk[ok]] + u[ok] // USE
    r_no[ok] = u[ok] % USE
    fix = np.where(~ok)[0]
    if len(fix) > FIX_CALLS * USE:
        raise RuntimeError(f"fixup overflow: {len(fix)}")
    fpos = np.arange(len(fix))
    call_no[fix] = N_MAIN + fpos // USE
    r_no[fix] = fpos % USE

    # gather position j = (r%7)*128 + r//7
    j_no = (r_no % WPP) * 128 + r_no // WPP

    # gather idx array [128, N_GATHER*ICOLS]: idx = ptab pair-row of the
    # node's ñ-rank: (ñ//392)*196 + (ñ%392)//2; parity bit = ñ%2
    idx_arr = np.zeros((128, N_GATHER * ICOLS), dtype=np.int16)
    fixe = np.where(call_no >= N_MAIN)[0]

    def pair_row(r):
        return (r // NCH) * QPP + (r % NCH) // 2

    tgt_vals = np.zeros((N_SLOT_CALLS, CALL), dtype=np.int64)
    tgt_vals[call_no, j_no] = pair_row(tv)
    src_vals = np.zeros((FIX_CALLS, CALL), dtype=np.int64)
    src_vals[call_no[fixe] - N_MAIN, j_no[fixe]] = pair_row(u[fixe])

    g = 0
    for ci in range(N_MAIN):
        idx_arr[:, g * ICOLS:(g + 1) * ICOLS] = _wrap_idx(tgt_vals[ci])
        g += 1
    for ci in range(FIX_CALLS):
        idx_arr[:, g * ICOLS:(g + 1) * ICOLS] = _wrap_idx(src_vals[ci])
        g += 1
    for ci in range(FIX_CALLS):
        idx_arr[:, g * ICOLS:(g + 1) * ICOLS] = _wrap_idx(
            tgt_vals[N_MAIN + ci])
        g += 1

    # attention pack [64, 8]
    a = np.empty((F_IN, 8), dtype=np.float16)
    a[:, :K] = att16[:, :F_IN].T
    a[:, K:] = att16[:, F_IN:].T

    # output location per edge: out_d[r//7, call*28 + (r%7)*4 + k]
    out_row = r_no // WPP
    out_col = call_no * (WPP * K) + (r_no % WPP) * K

    # parity masks, laid out like out_d columns (plus fixup-src block)
    mask = np.zeros((128, (N_SLOT_CALLS + FIX_CALLS) * WPP * K),
                    dtype=np.uint8)
    k4 = np.arange(K)[None, :]
    mask[out_row[:, None], out_col[:, None] + k4] = \
        (tv % 2).astype(np.uint8)[:, None]
    mask[out_row[fixe][:, None],
         out_col[fixe][:, None] + 2 * WPP * K + k4] = \
        (u[fixe] % 2).astype(np.uint8)[:, None]

    in_map = {"xT_in": xT, "a_in": a, "idx_in": idx_arr, "mask_in": mask}
    return in_map, out_row, out_col


def prepare_passes(x, edge_index, att):
    x16 = np.asarray(x, dtype=np.float32).astype(np.float16)
    att16 = np.asarray(att, dtype=np.float32).astype(np.float16)
    ei = np.asarray(edge_index).astype(np.int64)
    E_c = N_EDGES // CORES
    in_maps, maps = [], []
    for c in range(CORES):
        sl = slice(c * E_c, (c + 1) * E_c)
        # x16 rows must be passed in ñ order: prepare_core handles the
        # permutation internally via rank_of -> pass orig-order x
        im, orow, ocol = prepare_core(ei[0, sl], ei[1, sl], x16, att16)
        in_maps.append(im)
        maps.append((orow, ocol))
    return in_maps, maps


TRACE = False
LAST_RESULTS = []


def kernel(x, edge_index, att):
    nc = get_program()
    in_maps, maps = prepare_passes(x, edge_index, att)
    LAST_RESULTS.clear()
    res = run_bass_kernel_spmd(
        nc, in_maps, core_ids=list(range(CORES)), trace=TRACE)
    LAST_RESULTS.append(res)
    E_c = N_EDGES // CORES
    out = np.empty((N_EDGES, K), dtype=np.float32)
    for c in range(CORES):
        o = np.asarray(res.results[c]["out"])    # [128, cols] f16
        orow, ocol = maps[c]
        vals = o[orow[:, None], ocol[:, None] + np.arange(K)[None, :]]
        out[c * E_c:(c + 1) * E_c] = vals.astype(np.float32)
    return out


# revision 31
# speedup vs baseline: 1.3092x; 1.0522x over previous
"""Trainium2 Bass kernel for nn_MultiHeadLiftLayer (GNN edge-signal lift).

Computes, for each edge e with endpoints (s, t):
    out[e, k] = relu( x[s] . a_src[k] + x[t] . a_tgt[k] ),  k = 0..3

Architecture (v5, "rank-major expansion + single-side gather"):

The previous (baseline) kernel gathered both endpoints' x rows per edge
via SWDGE dma_gather; its trace shows the bottleneck is the Q7
descriptor-generation loop on the Pool engine (~2.5us per 896-idx call,
strictly serial) -- NOT DMA bandwidth. This version removes the entire
src side from the Q7 path:

  - Phase A (on device): p[n] = [x[n].a_src | x[n].a_tgt]  (8 f16 =
    16B per node) is computed by 391 node-major PE matmuls (lhsT =
    xT 128-node chunk, rhs = the 64x8 attention pack), cast f16 by the
    ACT engine, and stored to HBM as `p_plain` (contiguous 16B rows in
    a host-chosen node order) plus a 256B-strided copy `ptab` for the
    gather (SWDGE non-transpose gather requires a 256B-multiple row
    stride; element itself is 16B -- the bass-level 256B elem_size
    assert only applies to transpose mode, so we emit the instruction
    directly).
  - Node order ñ = per-core src-degree descending. Edge e is assigned
    slot (k = rank of e within its src node, u = ñ(s_e)). Slots are
    laid out rank-major: segment k holds nodes u < n_k (a PREFIX of ñ
    since ñ is degree-sorted). The src-side values for segment k are
    therefore a contiguous prefix of p_plain -- ONE affine 3-dim DMA
    per segment (no per-edge work at all).
  - The tgt side is the only per-edge gather: 16B rows from ptab via
    non-transpose dma_gather (idx j lands at partition j%128, word
    j//128 -- verified on HW). int16 indices cover all 50048 rows via a
    signed bias: the table AP is based at row 32768 and indices are
    ñ(t)-32768 (negative indices verified to address below the base on
    HW; CoreSim has an over-strict >=-1 assert, so sim mode is not
    supported for this kernel).
  - Calls carry 896 positions; position j=895 (slot r=895) is a
    structural pad so the trailing index of every call is >= 0 (the Q7
    ucode trims trailing negatives). A call covers 895 slots; slot r
    sits at (partition r//7, 16B-word r%7); gather position j =
    (r%7)*128 + r//7.
  - Combine: out = relu(DS[.., 0:4] + DT[.., 4:8]) with full
    128-partition parallelism (DVE add + ACT relu), f16, then one DMA
    per segment to HBM.
  - Edges that overflow the fixed per-segment capacities (src-rank >= 6,
    ~300 per core, or a segment fuller than mean+~8 sigma) go to 2
    fixup calls where BOTH endpoints are gathered; the fixup gathers are
    scheduled first so their combine chain hides under the main calls.
  - Gather instructions carry no explicit queue-spacing deps: the Q7
    ucode's own descriptor-ring await_space handles backpressure, so
    desc-gen runs at the serial Q7 floor (~2.0us/call) while the SDMA
    engines drain concurrently.

Measured: 409us vs 654us baseline (same rel err 5.7e-04). Remaining
profile: ~140us head (xT in + phase A + the 50K-descriptor respread --
fabric-bound, must complete before any gather), ~250us tgt-gather window
(Q7 desc-gen floor), ~20us tail.
"""

import numpy as np

import concourse.ap_utils as ap_utils
import concourse.bacc as bacc
import concourse.bass as bass
import concourse.mybir as mybir
import concourse.tile as tile
from concourse.bass_utils import run_bass_kernel_spmd
from concourse.instruction_name_ordered_set import InstructionNameOrderedSet

# ---- problem constants (hardcoded per contract) ----
N_NODES = 50000
N_EDGES = 800000
F_IN = 64
K = 4
CORES = 8

NP = 50176                 # padded node count = 128 * 392 (392 even:
                           # ñ-consecutive node PAIRS stay in-partition)
NCH = 392                  # node chunks of 128 (phase A matmuls)
QPP = NCH // 2             # node pairs per partition (196)
NPAIR = NP // 2            # ptab2 rows (25088, fits int16 unsigned-ish)
CALL = 896                 # gather positions per call
USE = 895                  # usable slots per call (pos 895 = pad)
WPP = 7                    # words per partition per call (896/128)

# fixed per-segment call capacities, k = 0..5 (per-core degrees are
# ~Poisson(2); capacities are mean + >=6.6 sigma). Edges with src-rank
# >= 6 (~300 per core) go to the fixup calls.
CALLS_K = [49, 34, 19, 9, 4, 2]
KMAX = len(CALLS_K)
FIX_CALLS = 2              # fixup slot-calls (each needs 2 gathers)
N_MAIN = sum(CALLS_K)      # 123 main (tgt-gather) calls
N_SLOT_CALLS = N_MAIN + FIX_CALLS          # 127 slot-calls
N_GATHER = N_MAIN + 2 * FIX_CALLS          # 131 gather instructions
ICOLS = CALL // 16         # 56 idx columns per call (wrapped layout)

F32 = mybir.dt.float32
F16 = mybir.dt.float16
I16 = mybir.dt.int16

_PROGRAM_CACHE = {}


def _dma_gather_small(g, out_ap, in_ap, idxs_ap, num_idxs, elem_size,
                      elem_step, queue_num):
    """nc.gpsimd.dma_gather, non-transpose, without the 256B elem_size
    assert (which is a transpose-mode restriction; HW decode only
    requires the row stride to be a 256B multiple)."""
    g._assert_queue_num(queue_num)
    assert idxs_ap.dtype == mybir.dt.int16
    assert in_ap.dtype == out_ap.dtype
    assert ap_utils.ap_is_contiguous(in_ap.ap[1:])
    assert ap_utils.ap_is_contiguous(out_ap.ap[1:])
    assert ap_utils.ap_is_contiguous(idxs_ap.ap[1:])
    assert in_ap.ap[-1][1] == out_ap.ap[-1][1] == elem_size
    assert out_ap.ap[0][1] * out_ap.ap[1][1] == CALL
    assert in_ap.ap[0][0] == elem_step
    stride_bytes = elem_step * mybir.dt.size(in_ap.dtype)
    stride_bytes_256 = stride_bytes // 256
    assert stride_bytes_256 * 256 == stride_bytes and stride_bytes_256 < 256
    _in_ap = g.lower_ap_dma(in_ap, for_custom_bir_dma=True)
    _idxs_ap = g.lower_ap(idxs_ap)
    _out_ap = g.lower_ap(out_ap)
    return g.add_instruction(
        mybir.InstDMAGatherAnt(
            name=g.bass.get_next_instruction_name(),
            ins=[*_in_ap, _idxs_ap,
                 g.lower_val_access(g.to_reg(num_idxs))],
            outs=[_out_ap],
            transpose=False,
            num_idxs=num_idxs,
            elem_size=elem_size,
            stride_bytes_256=stride_bytes_256,
            gen_mode=0,
            single_packet=False,
            queue_num=queue_num,
        )
    )


def _build_program():
    nc = bacc.Bacc("TRN2", num_swdge_queues=4)

    xT_in = nc.dram_tensor("xT_in", [F_IN, NP], F16, kind="ExternalInput")
    a_in = nc.dram_tensor("a_in", [F_IN, 8], F16, kind="ExternalInput")
    idx_in = nc.dram_tensor("idx_in", [128, N_GATHER * ICOLS], I16,
                            kind="ExternalInput")
    # parity masks (f16 0/1): tgt parity for every slot-call, then src
    # parity for the fixup calls; layout mirrors out_d's columns
    mask_in = nc.dram_tensor(
        "mask_in", [128, (N_SLOT_CALLS + FIX_CALLS) * WPP * K],
        mybir.dt.uint8, kind="ExternalInput")
    out_d = nc.dram_tensor("out", [128, N_SLOT_CALLS * WPP * K], F16,
                           kind="ExternalOutput")
    p_plain = nc.dram_tensor("p_plain", [128, NCH * 8], F16, kind="Internal")
    # paired table: row q holds nodes (2 per row within a partition's ñ
    # range): 32B payload at 256B stride -> half the respread descriptors
    ptab = nc.dram_tensor("ptab", [NPAIR, 128], F16, kind="Internal")

    # segment -> (first main call index, ncalls)
    seg_base = []
    b = 0
    for k in range(KMAX):
        seg_base.append(b)
        b += CALLS_K[k]

    with tile.TileContext(nc) as tc:
        with (
            tc.tile_pool(name="const", bufs=1) as cpool,
            tc.tile_pool(name="ps", bufs=2, space="PSUM") as ppool,
            tc.tile_pool(name="seg", bufs=1) as spool,
        ):
            a_raw = cpool.tile([F_IN, 8], F16)
            nc.sync.dma_start(out=a_raw[:], in_=a_in[:])
            a_sb = cpool.tile([F_IN, 8], F16)
            nc.vector.tensor_copy(out=a_sb[:], in_=a_raw[:])
            idx = cpool.tile([128, N_GATHER * ICOLS], I16)
            nc.sync.dma_start(out=idx[:], in_=idx_in[:])
            mtile = cpool.tile([128, (N_SLOT_CALLS + FIX_CALLS) * WPP * K],
                               mybir.dt.uint8)
            nc.sync.dma_start(out=mtile[:], in_=mask_in[:])
            # xT arrives per-supertile so matmuls/casts/respreads pipeline
            xt = cpool.tile([F_IN, NP], F16)
            xt_done = 0
            while xt_done < NCH:
                m = min(64, NCH - xt_done)
                nc.sync.dma_start(
                    out=xt[:, 128 * xt_done:128 * (xt_done + m)],
                    in_=xT_in[:, 128 * xt_done:128 * (xt_done + m)])
                xt_done += m

            # ---- Phase A: p = [x.a_src | x.a_tgt] per node ----
            # Per 64-chunk supertile: matmuls -> f16 cast -> (a) write to
            # p_plain (contiguous, 128 descs) and (b) respread straight
            # into ptab's 256B-strided rows. The respreads (50K 16B
            # descriptors total) pipeline under the remaining matmuls
            # instead of serializing before the gathers.
            stage = cpool.tile([128, NCH * 8], F16)
            done = 0
            while done < NCH:
                m = min(64, NCH - done)
                ps = ppool.tile([128, 8 * m], F32)
                for i in range(m):
                    c = done + i
                    nc.tensor.matmul(
                        out=ps[:, 8 * i:8 * i + 8],
                        lhsT=xt[:, 128 * c:128 * c + 128],
                        rhs=a_sb[:, 0:8],
                        start=True,
                        stop=True,
                    )
                sl = stage[:, 8 * done:8 * (done + m)]
                nc.scalar.copy(out=sl, in_=ps[:, 0:8 * m])
                nc.sync.dma_start(
                    out=bass.AP(p_plain, 8 * done, [[NCH * 8, 128], [1, 8 * m]]),
                    in_=sl)
                # ptab pair-rows q = p*196 + (done+c)/2, 32B payload each
                nc.sync.dma_start(
                    out=bass.AP(ptab, 128 * (done // 2),
                                [[QPP * 128, 128], [128, m // 2], [1, 16]]),
                    in_=sl.rearrange("p (q e) -> p q e", e=16))
                done += m

            # ---- segment tiles (DT holds 32B pair-rows per slot) ----
            ds_tiles, dt_tiles, ad_tiles, o_tiles = [], [], [], []
            for k in range(KMAX):
                ncal = CALLS_K[k]
                dst = spool.tile([128, ncal * WPP * 8], F16, tag=f"ds{k}")
                dtt = spool.tile([128, ncal * WPP * 16], F16, tag=f"dt{k}")
                adt = spool.tile([128, ncal * WPP * K], F16, tag=f"ad{k}")
                ott = spool.tile([128, ncal * WPP * K], F16, tag=f"o{k}")
                ds_tiles.append(dst)
                dt_tiles.append(dtt)
                ad_tiles.append(adt)
                o_tiles.append(ott)
            # fixup tiles
            dsf = spool.tile([128, FIX_CALLS * WPP * 16], F16, tag="dsf")
            dtf = spool.tile([128, FIX_CALLS * WPP * 16], F16, tag="dtf")
            adf = spool.tile([128, FIX_CALLS * WPP * K], F16, tag="adf")
            of = spool.tile([128, FIX_CALLS * WPP * K], F16, tag="of")

            # ---- tgt-side (and fixup src) gathers ----
            tab_ap = ptab[:, 0:16]
            all_g = []

            def gather(dst_tile, call_local, gidx):
                o = dst_tile[:, call_local * 112:(call_local + 1) * 112]
                gi = _dma_gather_small(
                    nc.gpsimd,
                    out_ap=o.rearrange("p (o m) -> p o m", o=WPP),
                    in_ap=tab_ap,
                    idxs_ap=idx[:, gidx * ICOLS:(gidx + 1) * ICOLS],
                    num_idxs=CALL,
                    elem_size=16,
                    elem_step=128,
                    queue_num=len(all_g) % 4,
                )
                if all_g:
                    ns = InstructionNameOrderedSet()
                    ns.add(all_g[-1].ins.name)
                    gi.ins.add_nosync_dependencies_from(ns)
                all_g.append(gi)

            # fixup gathers FIRST so their combine chain overlaps the main
            # gathers instead of trailing the whole kernel
            gidx = N_MAIN
            for cl in range(FIX_CALLS):      # fixup src gathers
                gather(dsf, cl, gidx)
                gidx += 1
            for cl in range(FIX_CALLS):      # fixup tgt gathers
                gather(dtf, cl, gidx)
                gidx += 1
            gidx = 0
            for k in range(KMAX):
                for cl in range(CALLS_K[k]):
                    gather(dt_tiles[k], cl, gidx)
                    gidx += 1

            # ---- src-side affine expansion (emitted after the gathers
            # so its fabric time drains during the gather window, not
            # before it) ----
            for k in range(KMAX):
                ncal = CALLS_K[k]
                # src AP: (p: 7 slots = 56 elems, call: 895 slots = 7160
                # elems, run: 56 elems) over p_plain's flat [NP*8] f16
                src = bass.AP(p_plain, 0,
                              [[56, 128], [7160, ncal], [1, 56]])
                dsv = ds_tiles[k][:].rearrange("p (cl e) -> p cl e", e=56)
                nc.sync.dma_start(out=dsv, in_=src)

            # ---- combine: where parity, overwrite the even-node half
            # with the odd-node half IN PLACE (copy_predicated is cheap;
            # a separate select would pay a pathological strided
            # tensor_copy), then add + relu
            def sel_half(dtt, ncal, mask_col, base_off):
                n_sl = ncal * WPP
                d16 = dtt[:].rearrange("p (s e) -> p s e", e=16)
                mv = mtile[:, mask_col:mask_col + n_sl * K].rearrange(
                    "p (s e) -> p s e", e=4)
                nc.vector.copy_predicated(
                    out=d16[:, :, base_off:base_off + 4], mask=mv,
                    data=d16[:, :, base_off + 8:base_off + 12])
                return d16[:, :, base_off:base_off + 4]

            for k, kb in enumerate(seg_base):
                ncal = CALLS_K[k]
                sv = sel_half(dt_tiles[k], ncal, kb * WPP * K, 4)
                v0 = ds_tiles[k][:].rearrange(
                    "p (s e) -> p s e", e=8)[:, :, 0:4]
                av = ad_tiles[k][:].rearrange("p (s e) -> p s e", e=4)
                nc.vector.tensor_add(out=av, in0=v0, in1=sv)
                nc.scalar.activation(
                    out=o_tiles[k][:], in_=ad_tiles[k][:],
                    func=mybir.ActivationFunctionType.Relu)
            # fixup: both sides gathered pair-rows; select each
            svt = sel_half(dtf, FIX_CALLS, N_MAIN * WPP * K, 4)
            svs = sel_half(dsf, FIX_CALLS, N_SLOT_CALLS * WPP * K, 0)
            adfv = adf[:].rearrange("p (s e) -> p s e", e=4)
            nc.vector.tensor_add(out=adfv, in0=svs, in1=svt)
            nc.scalar.activation(out=of[:], in_=adf[:],
                                 func=mybir.ActivationFunctionType.Relu)

            # ---- output DMAs ----
            col = 0
            for k in range(KMAX):
                w = CALLS_K[k] * WPP * K
                nc.sync.dma_start(out=out_d[:, col:col + w],
                                  in_=o_tiles[k][:])
                col += w
            w = FIX_CALLS * WPP * K
            nc.sync.dma_start(out=out_d[:, col:col + w], in_=of[:])

    # pin each gather's SWDGE queue to its scheduled completion-sem lane
    from concourse.tile_sem_assignment import PROC_NAME_TO_IDX
    lane_of = {PROC_NAME_TO_IDX[f"DMASW{i}"]: i for i in range(8)}
    for blk in nc.main_func.blocks:
        for inst in blk.instructions:
            if isinstance(inst, mybir.InstDMAGatherAnt):
                lane = lane_of.get(inst.bass_scheduled_proc)
                if lane is not None:
                    inst.queue_num = lane % 4

    nc.compile()
    return nc


def get_program():
    if "nc" not in _PROGRAM_CACHE:
        _PROGRAM_CACHE["nc"] = _build_program()
    return _PROGRAM_CACHE["nc"]


def _wrap_idx(vals):
    """Wrap a length-CALL idx vector for SWDGE: pos j -> [16g + j%16,
    j//16], replicated across the 8 gpsimd cores."""
    w = vals.reshape(ICOLS, 16).T.astype(np.int16)
    return np.tile(w, (8, 1))


def prepare_core(s, t, x16, att16):
    """Host marshaling for one core: node ordering, slot assignment,
    gather indices, input tensors, and the slot->edge output map."""
    E_c = len(s)
    d = np.bincount(s, minlength=N_NODES)
    order = np.argsort(-d, kind="stable")          # ñ -> orig node id
    rank_of = np.empty(N_NODES, dtype=np.int64)
    rank_of[order] = np.arange(N_NODES)

    # xT: node with ñ-rank u -> column 128*(u%392) + u//392, so that
    # p_plain row ñ (= p*392 + c for stage partition p chunk c) == u
    xT = np.zeros((F_IN, NP), dtype=np.float16)
    uu = np.arange(N_NODES)
    cols = 128 * (uu % NCH) + (uu // NCH)
    xT[:, cols] = x16[order].T                     # x rows in ñ order

    # per-edge src rank k
    o = np.argsort(s, kind="stable")
    so = s[o]
    starts = np.searchsorted(so, so)               # first pos of value
    kk = np.empty(E_c, dtype=np.int64)
    kk[o] = np.arange(E_c) - starts
    u = rank_of[s]
    tv = rank_of[t]

    # slot assignment
    call_no = np.full(E_c, -1, dtype=np.int64)
    r_no = np.full(E_c, -1, dtype=np.int64)
    seg_base = np.cumsum([0] + CALLS_K[:-1])
    ok = kk < KMAX
    capn = np.array([CALLS_K[k] * USE for k in range(KMAX)])
    ok &= u < capn[np.clip(kk, 0, KMAX - 1)]
    call_no[ok] = seg_base[kk[ok]] + u[ok] // USE
    r_no[ok] = u[ok] % USE
    fix = np.where(~ok)[0]
    if len(fix) > FIX_CALLS * USE:
        raise RuntimeError(f"fixup overflow: {len(fix)}")
    fpos = np.arange(len(fix))
    call_no[fix] = N_MAIN + fpos // USE
    r_no[fix] = fpos % USE

    # gather position j = (r%7)*128 + r//7
    j_no = (r_no % WPP) * 128 + r_no // WPP

    # gather idx array [128, N_GATHER*ICOLS]: idx = ptab pair-row of the
    # node's ñ-rank: (ñ//392)*196 + (ñ%392)//2; parity bit = ñ%2
    idx_arr = np.zeros((128, N_GATHER * ICOLS), dtype=np.int16)
    fixe = np.where(call_no >= N_MAIN)[0]

    def pair_row(r):
        return (r // NCH) * QPP + (r % NCH) // 2

    tgt_vals = np.zeros((N_SLOT_CALLS, CALL), dtype=np.int64)
    tgt_vals[call_no, j_no] = pair_row(tv)
    src_vals = np.zeros((FIX_CALLS, CALL), dtype=np.int64)
    src_vals[call_no[fixe] - N_MAIN, j_no[fixe]] = pair_row(u[fixe])

    g = 0
    for ci in range(N_MAIN):
        idx_arr[:, g * ICOLS:(g + 1) * ICOLS] = _wrap_idx(tgt_vals[ci])
        g += 1
    for ci in range(FIX_CALLS):
        idx_arr[:, g * ICOLS:(g + 1) * ICOLS] = _wrap_idx(src_vals[ci])
        g += 1
    for ci in range(FIX_CALLS):
        idx_arr[:, g * ICOLS:(g + 1) * ICOLS] = _wrap_idx(
            tgt_vals[N_MAIN + ci])
        g += 1

    # attention pack [64, 8]
    a = np.empty((F_IN, 8), dtype=np.float16)
    a[:, :K] = att16[:, :F_IN].T
    a[:, K:] = att16[:, F_IN:].T

    # output location per edge: out_d[r//7, call*28 + (r%7)*4 + k]
    out_row = r_no // WPP
    out_col = call_no * (WPP * K) + (r_no % WPP) * K

    # parity masks, laid out like out_d columns (plus fixup-src block)
    mask = np.zeros((128, (N_SLOT_CALLS + FIX_CALLS) * WPP * K),
                    dtype=np.uint8)
    k4 = np.arange(K)[None, :]
    mask[out_row[:, None], out_col[:, None] + k4] = \
        (tv % 2).astype(np.uint8)[:, None]
    mask[out_row[fixe][:, None],
         out_col[fixe][:, None] + 2 * WPP * K + k4] = \
        (u[fixe] % 2).astype(np.uint8)[:, None]

    in_map = {"xT_in": xT, "a_in": a, "idx_in": idx_arr, "mask_in": mask}
    return in_map, out_row, out_col


def prepare_passes(x, edge_index, att):
    x16 = np.asarray(x, dtype=np.float32).astype(np.float16)
    att16 = np.asarray(att, dtype=np.float32).astype(np.float16)
    ei = np.asarray(edge_index).astype(np.int64)
    E_c = N_EDGES // CORES
    in_maps, maps = [], []
    for c in range(CORES):
        sl = slice(c * E_c, (c + 1) * E_c)
        # x16 rows must be passed in ñ order: prepare_core handles the
        # permutation internally via rank_of -> pass orig-order x
        im, orow, ocol = prepare_core(ei[0, sl], ei[1, sl], x16, att16)
        in_maps.append(im)
        maps.append((orow, ocol))
    return in_maps, maps


TRACE = False
LAST_RESULTS = []


def kernel(x, edge_index, att):
    nc = get_program()
    in_maps, maps = prepare_passes(x, edge_index, att)
    LAST_RESULTS.clear()
    res = run_bass_kernel_spmd(
        nc, in_maps, core_ids=list(range(CORES)), trace=TRACE)
    LAST_RESULTS.append(res)
    E_c = N_EDGES // CORES
    out = np.empty((N_EDGES, K), dtype=np.float32)
    for c in range(CORES):
        o = np.asarray(res.results[c]["out"])    # [128, cols] f16
        orow, ocol = maps[c]
        vals = o[orow[:, None], ocol[:, None] + np.arange(K)[None, :]]
        out[c * E_c:(c + 1) * E_c] = vals.astype(np.float32)
    return out
